# revision 1
# baseline (speedup 1.0000x reference)
"""DiffJPEG decode kernel for Trainium2 (8 NeuronCores, batch-parallel).

Pipeline per image (validated in numpy against the reference, see sim.py):
  Y:  natural DMA load -> T_in (PE transpose) -> y-matmul (dequant+col-IDCT
      folded into per-image lhsT) -> T2 -> T3 (PE transposes that convert the
      block layout to image-row layout) -> x-matmul (row-IDCT) -> biased
      drains (color constants folded)
  C:  same front; then fused row-upsample+row-IDCT matmuls (color scales
      1.403/1.773/0.344/0.714 folded into constant lhsTs), col-upsample on
      DVE via shifted adds, color combine on DVE.

Layout bit-conventions (Y, n in [0,4096)):
  n = 256 t + 2 p + s   (t:16, p:128 partitions, s:2)
  unpatchify: a = t[3:2], ii = (t[1:0], p[6:5]), j = (p[4:0], s)
  row r = 128 a + 8 ii + u,  col c = 16 p[4:0] + 8 s + v
Chroma (n' in [0,1024)): n' = 256 t' + 2 p + s; a' = t'[1],
  ii' = (t'[0], p[6:4]), j' = (p[3:0], s).
"""
import os
import sys
import numpy as np

sys.path.insert(0, "/opt/trn_rl_repo")

import concourse.bass as bass
import concourse.mybir as mybir
import concourse.tile as tile
from concourse.tile import add_dep_helper
from concourse.bass_utils import run_bass_kernel_spmd
from concourse.masks import make_identity

F32 = mybir.dt.float32
F32R = mybir.dt.float32r
COPY = mybir.ActivationFunctionType.Copy

# ------------------------------------------------------------------ host math

QT_Y = np.array([[16,11,10,16,24,40,51,61],[12,12,14,19,26,58,60,55],[14,13,16,24,40,57,69,56],[14,17,22,29,51,87,80,62],[18,22,37,56,68,109,103,77],[24,35,55,64,81,104,113,92],[49,64,78,87,103,121,120,101],[72,92,95,98,112,100,103,99]], dtype=np.float32)
QT_C = np.array([[17,18,24,47,99,99,99,99],[18,21,26,66,99,99,99,99],[24,26,56,99,99,99,99,99],[47,66,99,99,99,99,99,99],[99,99,99,99,99,99,99,99],[99,99,99,99,99,99,99,99],[99,99,99,99,99,99,99,99],[99,99,99,99,99,99,99,99]], dtype=np.float32)

SCALE_CR2 = np.float32(1.403)
SCALE_CB2 = np.float32(1.773)
SCALE_GC_CB = np.float32(0.344)
SCALE_GC_CR = np.float32(0.714)
_K = np.float32(128.0 / 255.0)
_OFF = np.float32(128.0 / 255.0 - 0.5)
C_R = float(_K + SCALE_CR2 * _OFF)
C_G = float(_K - (SCALE_GC_CB + SCALE_GC_CR) * _OFF)
C_B = float(_K + SCALE_CB2 * _OFF)

# (b, half) windows with nonzero fused-upsample weight
UPS_WINDOWS = [(0, 0), (1, 0), (1, 1), (2, 0), (2, 1), (3, 1)]
# map sources: (name, [(channel, scale), ...]); channel 0=cb, 1=cr
UPS_MAPS = [("cr2", [(1, SCALE_CR2)]),
            ("cb2", [(0, SCALE_CB2)]),
            ("gc", [(0, SCALE_GC_CB), (1, SCALE_GC_CR)])]


def _poly_floor_np(x):
    f = np.floor(x)
    return (f + (x - np.float32(0.5) - f) ** 3).astype(np.float32)


def _diff_clip_np(x, mn, mx, scale=np.float32(0.02)):
    with np.errstate(over="ignore"):
        x = np.where(x > mx, -scale * (np.exp(-x + mx) - np.float32(1.0)) + mx, x)
        x = np.where(x < mn, scale * (np.exp(x - mn) - np.float32(1.0)) + mn, x)
    return x.astype(np.float32)


def dequant_factor(q, qt):
    q = q.astype(np.float32)
    s = _poly_floor_np(np.where(q < 50.0, np.float32(5000.0) / q, np.float32(200.0) - 2.0 * q))
    qts = qt[None, :, :] * s[:, None, None]
    return _poly_floor_np(_diff_clip_np((qts + np.float32(50.0)) / np.float32(100.0), np.float32(1.0), np.float32(255.0)))


def idct_A():
    x = np.arange(8, dtype=np.float64)
    u = np.arange(8, dtype=np.float64)
    alpha = np.ones(8, dtype=np.float64)
    alpha[0] = 1.0 / np.sqrt(2.0)
    A = 0.5 * alpha[:, None] * np.cos((2.0 * u[None, :] + 1.0) * x[:, None] * np.pi / 16.0)
    return A.astype(np.float32)


def upsample_U(n_in):
    n_out = 2 * n_in
    U = np.zeros((n_out, n_in), dtype=np.float32)
    for R in range(n_out):
        k, odd = divmod(R, 2)
        if odd:
            U[R, k] += 0.75
            U[R, min(k + 1, n_in - 1)] += 0.25
        else:
            U[R, k] += 0.75
            U[R, max(k - 1, 0)] += 0.25
    return U


def make_lhsT_y(F):
    """[128,128]: k=(s,x,y)->m=(s,x,v): F[x,y]*A[y,v]/255 (diag in s,x)."""
    A = idct_A()
    W = np.zeros((2, 8, 8, 2, 8, 8), dtype=np.float32)
    for s in range(2):
        for xx in range(8):
            W[s, xx, :, s, xx, :] = (F[xx, :, None] * A) / np.float32(255.0)
    return W.reshape(128, 128)


def make_lhsT_x():
    """[128,128]: k=(ii,x)->m=(ii,u): A[x,u] (diag in ii)."""
    A = idct_A()
    W = np.zeros((16, 8, 16, 8), dtype=np.float32)
    for ii in range(16):
        W[ii, :, ii, :] = A
    return W.reshape(128, 128)


def make_ups_lhsT(b, half, scale):
    """[128,128] fused row-upsample+row-IDCT for chroma, scaled."""
    A = idct_A()
    U = upsample_U(256)
    W = np.zeros((16, 8, 128), dtype=np.float32)
    for ii in range(16):
        ip = 16 * half + ii
        Ublk = U[128 * b:128 * (b + 1), 8 * ip:8 * ip + 8]
        W[ii] = np.float32(scale) * (A @ Ublk.T)
    return W.reshape(128, 128)


def host_consts(jpeg_quality, qt_y, qt_c):
    B = jpeg_quality.shape[0]
    Fy = dequant_factor(jpeg_quality, qt_y)
    Fc = dequant_factor(jpeg_quality, qt_c)
    lhsty = np.stack([make_lhsT_y(Fy[i]) for i in range(B)])
    lhstyc = np.stack([make_lhsT_y(Fc[i]) for i in range(B)])
    lhstx = make_lhsT_x()
    ups = np.zeros((3, 2, 6, 128, 128), dtype=np.float32)
    for mi, (name, srcs) in enumerate(UPS_MAPS):
        for ch, scale in srcs:
            for wi, (b, half) in enumerate(UPS_WINDOWS):
                ups[mi, ch, wi] = make_ups_lhsT(b, half, scale)
    return lhsty, lhstyc, lhstx, ups


# ------------------------------------------------------------------ device

def split_excess_waits(nc, max_waits=1):
    """Walrus caps sem-waits per instruction; hoist excess onto same-engine
    NOPs inserted immediately before (same sequencer => semantics equal)."""
    for f in nc.m.functions:
        for blk in f.blocks:
            insts = blk.instructions
            idx = 0
            while idx < len(insts):
                inst = insts[idx]
                si = inst.sync_info
                if si is not None and si.on_wait is not None and len(si.on_wait) > max_waits:
                    waits = list(si.on_wait)
                    keep = waits[-max_waits:]
                    excess = waits[:-max_waits]
                    pos = idx
                    for c0 in range(0, len(excess), max_waits):
                        chunk = excess[c0:c0 + max_waits]
                        nop = mybir.InstNoOp(name=nc.get_next_instruction_name(),
                                             engine=inst.engine, ins=[], outs=[])
                        nop.sync_info = mybir.SyncInfo(on_wait=chunk, on_update=[])
                        nc.register_instruction(nop)
                        insts.insert(pos, nop)
                        pos += 1
                        idx += 1
                    si.on_wait = keep
                idx += 1


IMGS_PER_CORE = 8
STAGES = int(os.environ.get("KERNEL_STAGES", "7"))
USE_F32R = True  # dtype for ymm/xmm/ups matmul inputs (transposes stay f32)
MMDT = F32R if USE_F32R else F32


def build_nc(reps=1):
    nc = bass.Bass()
    I = IMGS_PER_CORE

    wy_d = nc.dram_tensor("wy", [I, 4096, 64], F32, kind="ExternalInput")
    wcb_d = nc.dram_tensor("wcb", [I, 1024, 64], F32, kind="ExternalInput")
    wcr_d = nc.dram_tensor("wcr", [I, 1024, 64], F32, kind="ExternalInput")
    lhsty_d = nc.dram_tensor("lhsty", [I, 128, 128], F32, kind="ExternalInput")
    lhstyc_d = nc.dram_tensor("lhstyc", [I, 128, 128], F32, kind="ExternalInput")
    lhstx_d = nc.dram_tensor("lhstx", [128, 128], F32, kind="ExternalInput")
    ups_d = nc.dram_tensor("upsw", [3, 2, 6, 128, 128], F32, kind="ExternalInput")
    out_d = nc.dram_tensor("rgb", [reps, I, 3, 512, 512], F32, kind="ExternalOutput")

    with tile.TileContext(nc) as tc:
        for rep in range(reps):
            _build_body(nc, tc, wy_d, wcb_d, wcr_d, lhsty_d, lhstyc_d, lhstx_d,
                        ups_d, out_d[rep])
    split_excess_waits(nc)
    return nc


def _build_body(nc, tc, wy_d, wcb_d, wcr_d, lhsty_d, lhstyc_d, lhstx_d, ups_d, out_d):
    I = IMGS_PER_CORE
    tails = []

    def tail(inst, img):
        if img == I - 1:
            tails.append(inst)
        return inst

    with tc.tile_pool(name="const", bufs=1) as constp, \
         tc.tile_pool(name="ld", bufs=2) as ldp, \
         tc.tile_pool(name="mid", bufs=1) as midp, \
         tc.tile_pool(name="mid2", bufs=2) as midp2, \
         tc.tile_pool(name="outb", bufs=2) as outbp, \
         tc.tile_pool(name="ps", bufs=8, space="PSUM") as psp:

        ident = constp.tile([128, 128], F32, tag="ident")
        make_identity(nc, ident[:])

        # constant weights: DMA f32 then ACT-recast to matmul dtype (walrus
        # requires fp32r matmul inputs to be produced by a rounding op)
        lhstx_f = constp.tile([128, 128], F32, tag="lhstx_f")
        nc.sync.dma_start(out=lhstx_f[:], in_=lhstx_d[:])
        lhstx_t = constp.tile([128, 128], MMDT, tag="lhstx")
        nc.scalar.activation(out=lhstx_t[:], in_=lhstx_f[:], func=COPY)

        lhsty_f = constp.tile([128, 128 * I], F32, tag="lhsty_f")
        nc.sync.dma_start(out=lhsty_f[:].rearrange("k (i m) -> k i m", i=I),
                          in_=lhsty_d.rearrange("i k m -> k i m"))
        lhsty_t = constp.tile([128, 128 * I], MMDT, tag="lhsty")
        nc.scalar.activation(out=lhsty_t[:], in_=lhsty_f[:], func=COPY)

        lhstyc_f = constp.tile([128, 128 * I], F32, tag="lhstyc_f")
        nc.sync.dma_start(out=lhstyc_f[:].rearrange("k (i m) -> k i m", i=I),
                          in_=lhstyc_d.rearrange("i k m -> k i m"))
        lhstyc_t = constp.tile([128, 128 * I], MMDT, tag="lhstyc")
        nc.scalar.activation(out=lhstyc_t[:], in_=lhstyc_f[:], func=COPY)

        upsw_f = constp.tile([128, 3 * 2 * 6 * 128], F32, tag="upsw_f")
        nc.sync.dma_start(out=upsw_f[:].rearrange("k (m c w n) -> k m c w n", m=3, c=2, w=6),
                          in_=ups_d.rearrange("m c w k n -> k m c w n"))
        upsw_t = constp.tile([128, 3 * 2 * 6 * 128], MMDT, tag="upsw")
        nc.scalar.activation(out=upsw_t[:], in_=upsw_f[:], func=COPY)

        def upsw(mi, ch, wi):
            off = ((mi * 2 + ch) * 6 + wi) * 128
            return upsw_t[:, off:off + 128]

        for img in range(I):
            # ---------------- loads (8KB / 2KB contiguous runs) ----------
            xnat_y = ldp.tile([128, 2048], F32, tag="xnat_y")
            nc.sync.dma_start(out=xnat_y[:],
                              in_=wy_d[img].rearrange("(p w) c -> p (w c)", p=128, w=32))
            xnat_c = ldp.tile([128, 1024], F32, tag="xnat_c")
            nc.sync.dma_start(out=xnat_c[:, 0:512],
                              in_=wcb_d[img].rearrange("(p w) c -> p (w c)", p=128, w=8))
            nc.sync.dma_start(out=xnat_c[:, 512:1024],
                              in_=wcr_d[img].rearrange("(p w) c -> p (w c)", p=128, w=8))

            if STAGES < 1:
                dtile = outbp.tile([128, 512], F32, tag="r_t")
                nc.vector.tensor_copy(dtile[:], xnat_y[:, 0:512])
                nc.sync.dma_start(out=out_d[img, 0, 0:128, :], in_=dtile[:])
                continue
            # ---------------- T_in (Y): per-jw picks ---------------------
            # XT'[k=(s,x,y), f1 = ii*128 + a*32 + P0*16 + jw]
            xt_y = midp2.tile([128, 2048], MMDT, tag="xt_y")
            xt_y_r = xt_y[:].rearrange("k (ii a P0 jw) -> k ii a P0 jw",
                                       ii=16, a=4, P0=2, jw=16)
            for jq in range(4):
                p = psp.tile([128, 512], F32, tag="ps")
                for jl in range(4):
                    jw = 4 * jq + jl
                    nc.tensor.transpose(out=p[:, 128 * jl:128 * (jl + 1)],
                                        in_=xnat_y[:, 128 * jw:128 * (jw + 1)],
                                        identity=ident[:])
                srcr = p[:].rearrange("k (jl a ii P0) -> k jl a ii P0", jl=4, a=4, ii=16)
                for a in range(4):
                    dst = xt_y_r[:, :, a, :, 4 * jq:4 * (jq + 1)].rearrange(
                        "k ii P0 jl -> k jl ii P0")
                    nc.scalar.activation(out=dst, in_=srcr[:, :, a], func=COPY)

            # ---------------- T_in (C): per (ch, w3h) picks --------------
            # XT_c'[k, f1c = ii*64 + ch*32 + a*16 + P10*4 + w3h]
            xt_c = midp2.tile([128, 1024], MMDT, tag="xt_c")
            xt_c_r = xt_c[:].rearrange("k (ii ch a P10 w3h) -> k ii ch a P10 w3h",
                                       ii=16, ch=2, a=2, P10=4, w3h=4)
            for ch in range(2):
                p = psp.tile([128, 512], F32, tag="ps")
                for w3h in range(4):
                    base = 512 * ch + 128 * w3h
                    nc.tensor.transpose(out=p[:, 128 * w3h:128 * (w3h + 1)],
                                        in_=xnat_c[:, base:base + 128],
                                        identity=ident[:])
                srcr = p[:].rearrange("k (w3h a ii P10) -> k w3h a ii P10",
                                      w3h=4, a=2, ii=16)
                for a in range(2):
                    dst = xt_c_r[:, :, ch, a, :, :].rearrange(
                        "k ii P10 w3h -> k w3h ii P10")
                    nc.scalar.activation(out=dst, in_=srcr[:, :, a], func=COPY)

            if STAGES < 2:
                dtile = outbp.tile([128, 512], F32, tag="r_t")
                nc.vector.tensor_copy(dtile[:], xt_y[:, 0:512].bitcast(F32))
                nc.sync.dma_start(out=out_d[img, 0, 0:128, :], in_=dtile[:])
                continue
            # ---------------- y-matmul (contiguous drain) ----------------
            wimg_y = lhsty_t[:, 128 * img:128 * (img + 1)]
            zt_y = midp2.tile([128, 2048], F32, tag="zt_y")
            for c4 in range(4):
                p = psp.tile([128, 512], F32, tag="ps")
                nc.tensor.matmul(out=p[:], lhsT=wimg_y,
                                 rhs=xt_y[:, 512 * c4:512 * (c4 + 1)],
                                 start=True, stop=True)
                nc.scalar.activation(out=zt_y[:, 512 * c4:512 * (c4 + 1)], in_=p[:], func=COPY)
            wimg_c = lhstyc_t[:, 128 * img:128 * (img + 1)]
            zt_c = midp2.tile([128, 1024], F32, tag="zt_c")
            for c2 in range(2):
                p = psp.tile([128, 512], F32, tag="ps")
                nc.tensor.matmul(out=p[:], lhsT=wimg_c,
                                 rhs=xt_c[:, 512 * c2:512 * (c2 + 1)],
                                 start=True, stop=True)
                nc.scalar.activation(out=zt_c[:, 512 * c2:512 * (c2 + 1)], in_=p[:], func=COPY)

            if STAGES < 3:
                dtile = outbp.tile([128, 512], F32, tag="r_t")
                nc.vector.tensor_copy(dtile[:], zt_y[:, 0:512])
                nc.sync.dma_start(out=out_d[img, 0, 0:128, :], in_=dtile[:])
                continue
            # ---------------- T2 (Y): per-ii picks -----------------------
            # B3'[part=(a,P0,jw), f3' = s*1024 + v*128 + ii*8 + x]
            b3_y = midp2.tile([128, 2048], F32, tag="b3_y")
            b3_y_w = b3_y[:].rearrange("k (s v ii x) -> k s v ii x", s=2, v=8, ii=16, x=8)
            for iq in range(4):
                p = psp.tile([128, 512], F32, tag="ps")
                for il in range(4):
                    ii = 4 * iq + il
                    nc.tensor.transpose(out=p[:, 128 * il:128 * (il + 1)],
                                        in_=zt_y[:, 128 * ii:128 * (ii + 1)],
                                        identity=ident[:])
                srcr = p[:].rearrange("k (il s x v) -> k il s x v", il=4, s=2, x=8)
                for s in range(2):
                    dst = b3_y_w[:, s, :, 4 * iq:4 * (iq + 1), :].rearrange(
                        "k v il x -> k il x v")
                    nc.scalar.activation(out=dst, in_=srcr[:, :, s], func=COPY)

            # ---------------- T2 (C): per-ii 64-picks --------------------
            # B3c'[part=(ch,a,P10,w3h) 64, f3c' = s*1024 + v*128 + ii*8 + x]
            b3_c = midp2.tile([64, 2048], F32, tag="b3_c")
            b3_c_w = b3_c[:].rearrange("k (s v ii x) -> k s v ii x", s=2, v=8, ii=16, x=8)
            for iq in range(4):
                p = psp.tile([128, 512], F32, tag="ps")
                for il in range(4):
                    ii = 4 * iq + il
                    nc.tensor.transpose(out=p[0:64, 128 * il:128 * (il + 1)],
                                        in_=zt_c[:, 64 * ii:64 * (ii + 1)],
                                        identity=ident[:])
                srcr = p[0:64, :].rearrange("k (il s x v) -> k il s x v", il=4, s=2, x=8)
                for s in range(2):
                    dst = b3_c_w[:, s, :, 4 * iq:4 * (iq + 1), :].rearrange(
                        "k v il x -> k il x v")
                    nc.scalar.activation(out=dst, in_=srcr[:, :, s], func=COPY)

            if STAGES < 4:
                dtile = outbp.tile([128, 512], F32, tag="r_t")
                nc.vector.tensor_copy(dtile[:], b3_y[:, 0:512])
                nc.sync.dma_start(out=out_d[img, 0, 0:128, :], in_=dtile[:])
                continue
            # ---------------- T3 (Y): picks (ii, x) ----------------------
            # B4[part=(ii,x), f4 = a*512 + P0*256 + jw*16 + s*8 + v]
            b4_y = midp.tile([128, 2048], MMDT, tag="b4_y")
            b4_y_r = b4_y[:].rearrange("k (ap jw s v) -> k ap jw s v", ap=8, jw=16, s=2, v=8)
            for s in range(2):
                for vq in range(2):
                    p = psp.tile([128, 512], F32, tag="ps")
                    for vj in range(4):
                        v = vq * 4 + vj
                        base = s * 1024 + v * 128
                        nc.tensor.transpose(out=p[:, 128 * vj:128 * (vj + 1)],
                                            in_=b3_y[:, base:base + 128],
                                            identity=ident[:])
                    src = p[:].rearrange("k (vj ap jw) -> k vj ap jw", vj=4, ap=8)
                    dst = b4_y_r[:, :, :, s, vq * 4:(vq + 1) * 4].rearrange(
                        "k ap jw vj -> k vj ap jw")
                    nc.vector.tensor_copy(dst, src)

            # ---------------- T3 (C): picks (ii', x) ---------------------
            # B4c[part=(ii',x), f4c = ch*512 + a*256 + P10*64 + w3h*16 + s*8 + v]
            b4_c = midp.tile([128, 1024], MMDT, tag="b4_c")
            b4_c2 = b4_c[:].rearrange("k (ca pw s v) -> k ca pw s v", ca=4, pw=16, s=2, v=8)
            for s in range(2):
                for vq in range(2):
                    p = psp.tile([128, 512], F32, tag="ps")
                    for vj in range(4):
                        v = vq * 4 + vj
                        base = s * 1024 + v * 128
                        nc.tensor.transpose(out=p[:, 64 * vj:64 * (vj + 1)],
                                            in_=b3_c[0:64, base:base + 128],
                                            identity=ident[0:64, 0:64])
                    src = p[:, 0:256].rearrange("k (vj ca pw) -> k vj ca pw", vj=4, ca=4)
                    dst = b4_c2[:, :, :, s, vq * 4:(vq + 1) * 4].rearrange(
                        "k ca pw vj -> k vj ca pw")
                    nc.vector.tensor_copy(dst, src)

            if STAGES < 5:
                dtile = outbp.tile([128, 512], F32, tag="r_t")
                nc.vector.tensor_copy(dtile[:], b4_y[:, 0:512].bitcast(F32))
                nc.sync.dma_start(out=out_d[img, 0, 0:128, :], in_=dtile[:])
                continue
            # ---------------- x-matmul (Y) + drain -----------------------
            y_t = midp.tile([128, 2048], F32, tag="y_t")
            for a in range(4):
                p = psp.tile([128, 512], F32, tag="ps")
                nc.tensor.matmul(out=p[:], lhsT=lhstx_t[:],
                                 rhs=b4_y[:, 512 * a:512 * (a + 1)],
                                 start=True, stop=True)
                sl = slice(512 * a, 512 * (a + 1))
                nc.scalar.activation(out=y_t[:, sl], in_=p[:], func=COPY)

            if STAGES < 6:
                dtile = outbp.tile([128, 512], F32, tag="r_t")
                nc.vector.tensor_copy(dtile[:], y_t[:, 0:512])
                nc.sync.dma_start(out=out_d[img, 0, 0:128, :], in_=dtile[:])
                continue
            # ---------------- fused chroma ups matmuls + col-ups ---------
            for b in range(4):
                mps = []
                for mi, (name, srcs) in enumerate(UPS_MAPS):
                    p = psp.tile([128, 512], F32, tag="ps")
                    calls = []
                    for ch, scale in srcs:
                        for wi, (wb, half) in enumerate(UPS_WINDOWS):
                            if wb != b:
                                continue
                            calls.append((mi, ch, wi, half))
                    for idx, (mi2, ch, wi, half) in enumerate(calls):
                        rhs = b4_c[:, 512 * ch + 256 * half: 512 * ch + 256 * (half + 1)]
                        tail(nc.tensor.matmul(out=p[:, 0:256], lhsT=upsw(mi2, ch, wi),
                                              rhs=rhs, start=(idx == 0),
                                              stop=(idx == len(calls) - 1)), img)
                    mps.append(p)

                ups_sb = []
                map_bias = (C_R, C_B, -C_G)
                for mi, p in enumerate(mps):
                    q3 = outbp.tile([128, 256], F32, tag="q3")
                    q1 = outbp.tile([128, 256], F32, tag="q1")
                    tail(nc.scalar.activation(out=q3[:], in_=p[:, 0:256], func=COPY,
                                              scale=0.75, bias=0.75 * map_bias[mi]), img)
                    tail(nc.scalar.activation(out=q1[:], in_=p[:, 0:256], func=COPY,
                                              scale=0.25, bias=0.25 * map_bias[mi]), img)
                    m_up = outbp.tile([128, 512], F32, tag=f"mup_{mi}")
                    m2 = m_up[:].rearrange("k (c two) -> k c two", two=2)
                    nc.vector.tensor_add(m2[:, 1:256, 0], q3[:, 1:256], q1[:, 0:255])
                    nc.vector.tensor_add(m2[:, 0:255, 1], q3[:, 0:255], q1[:, 1:256])
                    nc.vector.tensor_add(m_up[:, 0:1], q3[:, 0:1], q1[:, 0:1])
                    tail(nc.vector.tensor_add(m_up[:, 511:512], q3[:, 255:256], q1[:, 255:256]), img)
                    ups_sb.append(m_up)

                # ---------------- color combine + store ------------------
                sl = slice(512 * b, 512 * (b + 1))
                r_t = outbp.tile([128, 512], F32, tag="r_t")
                g_t = outbp.tile([128, 512], F32, tag="g_t")
                bl_t = outbp.tile([128, 512], F32, tag="bl_t")
                tail(nc.vector.tensor_add(r_t[:], y_t[:, sl], ups_sb[0][:]), img)
                tail(nc.vector.tensor_sub(g_t[:], y_t[:, sl], ups_sb[2][:]), img)
                tail(nc.vector.tensor_add(bl_t[:], y_t[:, sl], ups_sb[1][:]), img)
                rows = slice(128 * b, 128 * (b + 1))
                if STAGES >= 7 or b == 0:
                    tail(nc.sync.dma_start(out=out_d[img, 0, rows, :], in_=r_t[:]), img)
                if STAGES >= 7:
                    tail(nc.sync.dma_start(out=out_d[img, 1, rows, :], in_=g_t[:]), img)
                    tail(nc.sync.dma_start(out=out_d[img, 2, rows, :], in_=bl_t[:]), img)

        # tail absorb: make SP observe all pending ticks so the final Tile
        # drain needs <=2 sem waits (walrus CTRL-queue cap)
        for prod in tails:
            n = nc.sync.nop()
            add_dep_helper(n.ins, prod.ins, sync=True, reason="tail absorb")


# ------------------------------------------------------------------ entry

_NC_CACHE = {}


def kernel(input_y, input_cb, input_cr, jpeg_quality,
           quantization_table_y, quantization_table_c, H, W):
    input_y = np.ascontiguousarray(np.asarray(input_y), dtype=np.float32)
    input_cb = np.ascontiguousarray(np.asarray(input_cb), dtype=np.float32)
    input_cr = np.ascontiguousarray(np.asarray(input_cr), dtype=np.float32)
    q = np.asarray(jpeg_quality, dtype=np.float32)
    qt_y = np.asarray(quantization_table_y, dtype=np.float32).reshape(8, 8)
    qt_c = np.asarray(quantization_table_c, dtype=np.float32).reshape(8, 8)
    B = input_y.shape[0]
    assert int(H) == 512 and int(W) == 512 and B == 64

    if "nc" not in _NC_CACHE:
        _NC_CACHE["nc"] = build_nc()
    nc = _NC_CACHE["nc"]

    n_cores = 8
    in_maps = _prep_in_maps(input_y, input_cb, input_cr, q, qt_y, qt_c, n_cores)
    res = run_bass_kernel_spmd(nc, in_maps, list(range(n_cores)))
    out = np.concatenate([res.results[c]["rgb"][0] for c in range(n_cores)], axis=0)
    return out.astype(np.float32)


def _prep_in_maps(input_y, input_cb, input_cr, q, qt_y, qt_c, n_cores=8):
    lhsty, lhstyc, lhstx, ups = host_consts(q, qt_y, qt_c)
    B = input_y.shape[0]
    per = B // n_cores
    in_maps = []
    for c in range(n_cores):
        sl = slice(c * per, (c + 1) * per)
        in_maps.append({
            "wy": input_y[sl].reshape(per, 4096, 64),
            "wcb": input_cb[sl].reshape(per, 1024, 64),
            "wcr": input_cr[sl].reshape(per, 1024, 64),
            "lhsty": lhsty[sl],
            "lhstyc": lhstyc[sl],
            "lhstx": lhstx,
            "upsw": ups,
        })
    return in_maps


def _make_sharded(nc, in_maps):
    import jax
    from jax.sharding import Mesh, PartitionSpec
    from jax.experimental.shard_map import shard_map
    from concourse import bass2jax, mybir as mb

    n_cores = len(in_maps)
    partition_name = nc.partition_id_tensor.name if nc.partition_id_tensor else None
    in_names, out_names, out_avals, zero_outs = [], [], [], []
    for alloc in nc.m.functions[0].allocations:
        if not isinstance(alloc, mb.MemoryLocationSet):
            continue
        name = alloc.memorylocations[0].name
        if alloc.kind == "ExternalInput":
            if name != partition_name:
                in_names.append(name)
        elif alloc.kind == "ExternalOutput":
            shape = tuple(alloc.tensor_shape)
            dtype = mb.dt.np(alloc.dtype)
            out_names.append(name)
            out_avals.append(jax.core.ShapedArray(shape, dtype))
            zero_outs.append(np.zeros(shape, dtype))
    n_params = len(in_names)
    all_in = in_names + out_names + ([partition_name] if partition_name else [])

    def _body(*args):
        operands = list(args)
        if partition_name is not None:
            operands.append(bass2jax.partition_id_tensor())
        outs = bass2jax._bass_exec_p.bind(
            *operands, out_avals=tuple(out_avals), in_names=tuple(all_in),
            out_names=tuple(out_names), lowering_input_output_aliases=(),
            sim_require_finite=True, sim_require_nnan=True, nc=nc)
        return tuple(outs)

    devices = jax.devices()[:n_cores]
    mesh = Mesh(np.asarray(devices), ("core",))
    nin = n_params + len(out_names)
    sharded = jax.jit(
        shard_map(_body, mesh=mesh, in_specs=(PartitionSpec("core"),) * nin,
                  out_specs=(PartitionSpec("core"),) * len(out_names),
                  check_rep=False),
        keep_unused=True)
    concat_in = [np.concatenate([np.asarray(in_maps[c][nm]) for c in range(n_cores)], axis=0)
                 for nm in in_names]
    concat_zero = [np.zeros((n_cores * z.shape[0], *z.shape[1:]), z.dtype) for z in zero_outs]
    dev_in = [jax.device_put(a) for a in concat_in + concat_zero]
    return sharded, dev_in


def time_kernel(inputs, reps=16, program_reps=5):
    """Estimate per-batch (64-image) exec ns via repeat-program differencing:
    exec = (T(program_reps) - T(1)) / (program_reps - 1); RPC overheads cancel."""
    import jax
    import time as _t
    from concourse import bass2jax

    bass2jax.install_neuronx_cc_hook()
    input_y = np.ascontiguousarray(np.asarray(inputs["input_y"]), dtype=np.float32)
    input_cb = np.ascontiguousarray(np.asarray(inputs["input_cb"]), dtype=np.float32)
    input_cr = np.ascontiguousarray(np.asarray(inputs["input_cr"]), dtype=np.float32)
    q = np.asarray(inputs["jpeg_quality"], dtype=np.float32)
    qt_y = np.asarray(inputs["quantization_table_y"], dtype=np.float32).reshape(8, 8)
    qt_c = np.asarray(inputs["quantization_table_c"], dtype=np.float32).reshape(8, 8)
    in_maps = _prep_in_maps(input_y, input_cb, input_cr, q, qt_y, qt_c)

    def bench(prog_reps):
        key = f"nc{prog_reps}"
        if key not in _NC_CACHE:
            _NC_CACHE[key] = build_nc(reps=prog_reps)
        sharded, dev_in = _make_sharded(_NC_CACHE[key], in_maps)
        jax.block_until_ready(sharded(*dev_in))  # warm
        times = []
        for _ in range(reps):
            t0 = _t.time()
            jax.block_until_ready(sharded(*dev_in))
            times.append(_t.time() - t0)
        return min(times), sorted(times)[len(times) // 2]

    t1_min, t1_med = bench(1)
    tR_min, tR_med = bench(program_reps)
    per_min = (tR_min - t1_min) / (program_reps - 1)
    per_med = (tR_med - t1_med) / (program_reps - 1)
    print(f"  T(1) min/med: {t1_min*1e3:.2f}/{t1_med*1e3:.2f} ms; "
          f"T({program_reps}) min/med: {tR_min*1e3:.2f}/{tR_med*1e3:.2f} ms")
    print(f"  per-batch exec: min-diff {per_min*1e6:.1f} us, med-diff {per_med*1e6:.1f} us")
    return per_med * 1e9


if __name__ == "__main__":
    rng = np.random.default_rng(0)
    B = 64
    inputs = dict(
        input_y=(rng.standard_normal((B, 4096, 8, 8)) * 10).astype(np.float32),
        input_cb=(rng.standard_normal((B, 1024, 8, 8)) * 10).astype(np.float32),
        input_cr=(rng.standard_normal((B, 1024, 8, 8)) * 10).astype(np.float32),
        jpeg_quality=rng.uniform(10, 95, size=B).astype(np.float32),
        quantization_table_y=QT_Y[None],
        quantization_table_c=QT_C[None],
        H=512, W=512,
    )
    out = kernel(**inputs)
    print("out", out.shape, out.dtype, float(np.abs(out).max()))



# revision 3
# speedup vs baseline: 12.5484x; 12.5484x over previous
"""DiffJPEG decode kernel for Trainium2 (8 NeuronCores, batch-parallel).

Pipeline per image (validated in numpy against the reference, see sim.py):
  Y:  natural DMA load -> T_in (PE transpose) -> y-matmul (dequant+col-IDCT
      folded into per-image lhsT) -> T2 -> T3 (PE transposes that convert the
      block layout to image-row layout) -> x-matmul (row-IDCT) -> biased
      drains (color constants folded)
  C:  same front; then fused row-upsample+row-IDCT matmuls (color scales
      1.403/1.773/0.344/0.714 folded into constant lhsTs), col-upsample on
      DVE via shifted adds, color combine on DVE.

Layout bit-conventions (Y, n in [0,4096)):
  n = 256 t + 2 p + s   (t:16, p:128 partitions, s:2)
  unpatchify: a = t[3:2], ii = (t[1:0], p[6:5]), j = (p[4:0], s)
  row r = 128 a + 8 ii + u,  col c = 16 p[4:0] + 8 s + v
Chroma (n' in [0,1024)): n' = 256 t' + 2 p + s; a' = t'[1],
  ii' = (t'[0], p[6:4]), j' = (p[3:0], s).
"""
import os
import sys
import numpy as np

sys.path.insert(0, "/opt/trn_rl_repo")

import concourse.bass as bass
import concourse.mybir as mybir
import concourse.tile as tile
from concourse.tile import add_dep_helper
from concourse.bass_utils import run_bass_kernel_spmd
from concourse.masks import make_identity

F32 = mybir.dt.float32
F32R = mybir.dt.float32r
COPY = mybir.ActivationFunctionType.Copy

# ------------------------------------------------------------------ host math

QT_Y = np.array([[16,11,10,16,24,40,51,61],[12,12,14,19,26,58,60,55],[14,13,16,24,40,57,69,56],[14,17,22,29,51,87,80,62],[18,22,37,56,68,109,103,77],[24,35,55,64,81,104,113,92],[49,64,78,87,103,121,120,101],[72,92,95,98,112,100,103,99]], dtype=np.float32)
QT_C = np.array([[17,18,24,47,99,99,99,99],[18,21,26,66,99,99,99,99],[24,26,56,99,99,99,99,99],[47,66,99,99,99,99,99,99],[99,99,99,99,99,99,99,99],[99,99,99,99,99,99,99,99],[99,99,99,99,99,99,99,99],[99,99,99,99,99,99,99,99]], dtype=np.float32)

SCALE_CR2 = np.float32(1.403)
SCALE_CB2 = np.float32(1.773)
SCALE_GC_CB = np.float32(0.344)
SCALE_GC_CR = np.float32(0.714)
_K = np.float32(128.0 / 255.0)
_OFF = np.float32(128.0 / 255.0 - 0.5)
C_R = float(_K + SCALE_CR2 * _OFF)
C_G = float(_K - (SCALE_GC_CB + SCALE_GC_CR) * _OFF)
C_B = float(_K + SCALE_CB2 * _OFF)

# (b, half) windows with nonzero fused-upsample weight
UPS_WINDOWS = [(0, 0), (1, 0), (1, 1), (2, 0), (2, 1), (3, 1)]
# map sources: (name, [(channel, scale), ...]); channel 0=cb, 1=cr
UPS_MAPS = [("cr2", [(1, SCALE_CR2)]),
            ("cb2", [(0, SCALE_CB2)]),
            ("gc", [(0, SCALE_GC_CB), (1, SCALE_GC_CR)])]


def _poly_floor_np(x):
    f = np.floor(x)
    return (f + (x - np.float32(0.5) - f) ** 3).astype(np.float32)


def _diff_clip_np(x, mn, mx, scale=np.float32(0.02)):
    with np.errstate(over="ignore"):
        x = np.where(x > mx, -scale * (np.exp(-x + mx) - np.float32(1.0)) + mx, x)
        x = np.where(x < mn, scale * (np.exp(x - mn) - np.float32(1.0)) + mn, x)
    return x.astype(np.float32)


def dequant_factor(q, qt):
    q = q.astype(np.float32)
    s = _poly_floor_np(np.where(q < 50.0, np.float32(5000.0) / q, np.float32(200.0) - 2.0 * q))
    qts = qt[None, :, :] * s[:, None, None]
    return _poly_floor_np(_diff_clip_np((qts + np.float32(50.0)) / np.float32(100.0), np.float32(1.0), np.float32(255.0)))


def idct_A():
    x = np.arange(8, dtype=np.float64)
    u = np.arange(8, dtype=np.float64)
    alpha = np.ones(8, dtype=np.float64)
    alpha[0] = 1.0 / np.sqrt(2.0)
    A = 0.5 * alpha[:, None] * np.cos((2.0 * u[None, :] + 1.0) * x[:, None] * np.pi / 16.0)
    return A.astype(np.float32)


def upsample_U(n_in):
    n_out = 2 * n_in
    U = np.zeros((n_out, n_in), dtype=np.float32)
    for R in range(n_out):
        k, odd = divmod(R, 2)
        if odd:
            U[R, k] += 0.75
            U[R, min(k + 1, n_in - 1)] += 0.25
        else:
            U[R, k] += 0.75
            U[R, max(k - 1, 0)] += 0.25
    return U


def make_lhsT_y(F):
    """[128,128]: k=(s,x,y)->m=(s,x,v): F[x,y]*A[y,v]/255 (diag in s,x)."""
    A = idct_A()
    W = np.zeros((2, 8, 8, 2, 8, 8), dtype=np.float32)
    for s in range(2):
        for xx in range(8):
            W[s, xx, :, s, xx, :] = (F[xx, :, None] * A) / np.float32(255.0)
    return W.reshape(128, 128)


def make_lhsT_x():
    """[128,128]: k=(ii,x)->m=(ii,u): A[x,u] (diag in ii)."""
    A = idct_A()
    W = np.zeros((16, 8, 16, 8), dtype=np.float32)
    for ii in range(16):
        W[ii, :, ii, :] = A
    return W.reshape(128, 128)


def make_ups_lhsT(b, half, scale):
    """[128,128] fused row-upsample+row-IDCT for chroma, scaled."""
    A = idct_A()
    U = upsample_U(256)
    W = np.zeros((16, 8, 128), dtype=np.float32)
    for ii in range(16):
        ip = 16 * half + ii
        Ublk = U[128 * b:128 * (b + 1), 8 * ip:8 * ip + 8]
        W[ii] = np.float32(scale) * (A @ Ublk.T)
    return W.reshape(128, 128)


def host_consts(jpeg_quality, qt_y, qt_c):
    B = jpeg_quality.shape[0]
    Fy = dequant_factor(jpeg_quality, qt_y)
    Fc = dequant_factor(jpeg_quality, qt_c)
    lhsty = np.stack([make_lhsT_y(Fy[i]) for i in range(B)])
    lhstyc = np.stack([make_lhsT_y(Fc[i]) for i in range(B)])
    lhstx = make_lhsT_x()
    ups = np.zeros((3, 2, 6, 128, 128), dtype=np.float32)
    for mi, (name, srcs) in enumerate(UPS_MAPS):
        for ch, scale in srcs:
            for wi, (b, half) in enumerate(UPS_WINDOWS):
                ups[mi, ch, wi] = make_ups_lhsT(b, half, scale)
    return lhsty, lhstyc, lhstx, ups


# ------------------------------------------------------------------ device

def split_excess_waits(nc, max_waits=1):
    """Walrus caps sem-waits per instruction; hoist excess onto same-engine
    NOPs inserted immediately before (same sequencer => semantics equal)."""
    for f in nc.m.functions:
        for blk in f.blocks:
            insts = blk.instructions
            idx = 0
            while idx < len(insts):
                inst = insts[idx]
                si = inst.sync_info
                if si is not None and si.on_wait is not None and len(si.on_wait) > max_waits:
                    waits = list(si.on_wait)
                    keep = waits[-max_waits:]
                    excess = waits[:-max_waits]
                    pos = idx
                    for c0 in range(0, len(excess), max_waits):
                        chunk = excess[c0:c0 + max_waits]
                        nop = mybir.InstNoOp(name=nc.get_next_instruction_name(),
                                             engine=inst.engine, ins=[], outs=[])
                        nop.sync_info = mybir.SyncInfo(on_wait=chunk, on_update=[])
                        nc.register_instruction(nop)
                        insts.insert(pos, nop)
                        pos += 1
                        idx += 1
                    si.on_wait = keep
                idx += 1


IMGS_PER_CORE = 8
STAGES = int(os.environ.get("KERNEL_STAGES", "7"))
ALIAS_OUT = bool(int(os.environ.get("KERNEL_ALIAS_OUT", "0")))
USE_F32R = True  # dtype for ymm/xmm/ups matmul inputs (transposes stay f32)
MMDT = F32R if USE_F32R else F32


def build_nc(reps=1):
    nc = bass.Bass()
    I = IMGS_PER_CORE

    wy_d = nc.dram_tensor("wy", [I, 4096, 64], F32, kind="ExternalInput")
    wcb_d = nc.dram_tensor("wcb", [I, 1024, 64], F32, kind="ExternalInput")
    wcr_d = nc.dram_tensor("wcr", [I, 1024, 64], F32, kind="ExternalInput")
    lhsty_d = nc.dram_tensor("lhsty", [I, 128, 128], F32, kind="ExternalInput")
    lhstyc_d = nc.dram_tensor("lhstyc", [I, 128, 128], F32, kind="ExternalInput")
    lhstx_d = nc.dram_tensor("lhstx", [128, 128], F32, kind="ExternalInput")
    ups_d = nc.dram_tensor("upsw", [3, 2, 6, 128, 128], F32, kind="ExternalInput")
    nout = 1 if ALIAS_OUT else reps
    out_d = nc.dram_tensor("rgb", [nout, I, 3, 512, 512], F32, kind="ExternalOutput")

    with tile.TileContext(nc) as tc:
        for rep in range(reps):
            _build_body(nc, tc, wy_d, wcb_d, wcr_d, lhsty_d, lhstyc_d, lhstx_d,
                        ups_d, out_d[0 if ALIAS_OUT else rep])
    split_excess_waits(nc)
    return nc


def _build_body(nc, tc, wy_d, wcb_d, wcr_d, lhsty_d, lhstyc_d, lhstx_d, ups_d, out_d):
    I = IMGS_PER_CORE
    tails = []

    def tail(inst, img):
        if img == I - 1:
            tails.append(inst)
        return inst

    with tc.tile_pool(name="const", bufs=1) as constp, \
         tc.tile_pool(name="ld", bufs=2) as ldp, \
         tc.tile_pool(name="mid", bufs=1) as midp, \
         tc.tile_pool(name="mid2", bufs=2) as midp2, \
         tc.tile_pool(name="outb", bufs=2) as outbp, \
         tc.tile_pool(name="ps", bufs=8, space="PSUM") as psp:

        ident = constp.tile([128, 128], F32, tag="ident")
        make_identity(nc, ident[:])

        # constant weights: DMA f32 then ACT-recast to matmul dtype (walrus
        # requires fp32r matmul inputs to be produced by a rounding op)
        lhstx_f = constp.tile([128, 128], F32, tag="lhstx_f")
        nc.sync.dma_start(out=lhstx_f[:], in_=lhstx_d[:])
        lhstx_t = constp.tile([128, 128], MMDT, tag="lhstx")
        nc.scalar.activation(out=lhstx_t[:], in_=lhstx_f[:], func=COPY)

        lhsty_f = constp.tile([128, 128 * I], F32, tag="lhsty_f")
        nc.sync.dma_start(out=lhsty_f[:].rearrange("k (i m) -> k i m", i=I),
                          in_=lhsty_d.rearrange("i k m -> k i m"))
        lhsty_t = constp.tile([128, 128 * I], MMDT, tag="lhsty")
        nc.scalar.activation(out=lhsty_t[:], in_=lhsty_f[:], func=COPY)

        lhstyc_f = constp.tile([128, 128 * I], F32, tag="lhstyc_f")
        nc.sync.dma_start(out=lhstyc_f[:].rearrange("k (i m) -> k i m", i=I),
                          in_=lhstyc_d.rearrange("i k m -> k i m"))
        lhstyc_t = constp.tile([128, 128 * I], MMDT, tag="lhstyc")
        nc.scalar.activation(out=lhstyc_t[:], in_=lhstyc_f[:], func=COPY)

        upsw_f = constp.tile([128, 3 * 2 * 6 * 128], F32, tag="upsw_f")
        nc.sync.dma_start(out=upsw_f[:].rearrange("k (m c w n) -> k m c w n", m=3, c=2, w=6),
                          in_=ups_d.rearrange("m c w k n -> k m c w n"))
        upsw_t = constp.tile([128, 3 * 2 * 6 * 128], MMDT, tag="upsw")
        nc.scalar.activation(out=upsw_t[:], in_=upsw_f[:], func=COPY)

        def upsw(mi, ch, wi):
            off = ((mi * 2 + ch) * 6 + wi) * 128
            return upsw_t[:, off:off + 128]

        for img in range(I):
            # ---------------- loads (8KB / 2KB contiguous runs) ----------
            xnat_y = ldp.tile([128, 2048], F32, tag="xnat_y")
            nc.sync.dma_start(out=xnat_y[:],
                              in_=wy_d[img].rearrange("(p w) c -> p (w c)", p=128, w=32))
            xnat_c = ldp.tile([128, 1024], F32, tag="xnat_c")
            nc.sync.dma_start(out=xnat_c[:, 0:512],
                              in_=wcb_d[img].rearrange("(p w) c -> p (w c)", p=128, w=8))
            nc.sync.dma_start(out=xnat_c[:, 512:1024],
                              in_=wcr_d[img].rearrange("(p w) c -> p (w c)", p=128, w=8))

            if STAGES < 1:
                dtile = outbp.tile([128, 512], F32, tag="r_t")
                nc.vector.tensor_copy(dtile[:], xnat_y[:, 0:512])
                nc.sync.dma_start(out=out_d[img, 0, 0:128, :], in_=dtile[:])
                continue
            # ---------------- T_in (Y): per-jw picks ---------------------
            # XT'[k=(s,x,y), f1 = ii*128 + a*32 + P0*16 + jw]
            xt_y = midp2.tile([128, 2048], MMDT, tag="xt_y")
            xt_y_r = xt_y[:].rearrange("k (ii a P0 jw) -> k ii a P0 jw",
                                       ii=16, a=4, P0=2, jw=16)
            for jq in range(4):
                p = psp.tile([128, 512], F32, tag="ps")
                for jl in range(4):
                    jw = 4 * jq + jl
                    nc.tensor.transpose(out=p[:, 128 * jl:128 * (jl + 1)],
                                        in_=xnat_y[:, 128 * jw:128 * (jw + 1)],
                                        identity=ident[:])
                srcr = p[:].rearrange("k (jl a ii P0) -> k jl a ii P0", jl=4, a=4, ii=16)
                for a in range(4):
                    dst = xt_y_r[:, :, a, :, 4 * jq:4 * (jq + 1)].rearrange(
                        "k ii P0 jl -> k jl ii P0")
                    nc.scalar.activation(out=dst, in_=srcr[:, :, a], func=COPY)

            # ---------------- T_in (C): per (ch, w3h) picks --------------
            # XT_c'[k, f1c = ii*64 + ch*32 + a*16 + P10*4 + w3h]
            xt_c = midp2.tile([128, 1024], MMDT, tag="xt_c")
            xt_c_r = xt_c[:].rearrange("k (ii ch a P10 w3h) -> k ii ch a P10 w3h",
                                       ii=16, ch=2, a=2, P10=4, w3h=4)
            for ch in range(2):
                p = psp.tile([128, 512], F32, tag="ps")
                for w3h in range(4):
                    base = 512 * ch + 128 * w3h
                    nc.tensor.transpose(out=p[:, 128 * w3h:128 * (w3h + 1)],
                                        in_=xnat_c[:, base:base + 128],
                                        identity=ident[:])
                srcr = p[:].rearrange("k (w3h a ii P10) -> k w3h a ii P10",
                                      w3h=4, a=2, ii=16)
                for a in range(2):
                    dst = xt_c_r[:, :, ch, a, :, :].rearrange(
                        "k ii P10 w3h -> k w3h ii P10")
                    nc.scalar.activation(out=dst, in_=srcr[:, :, a], func=COPY)

            if STAGES < 2:
                dtile = outbp.tile([128, 512], F32, tag="r_t")
                nc.vector.tensor_copy(dtile[:], xt_y[:, 0:512].bitcast(F32))
                nc.sync.dma_start(out=out_d[img, 0, 0:128, :], in_=dtile[:])
                continue
            # ---------------- y-matmul (contiguous drain) ----------------
            wimg_y = lhsty_t[:, 128 * img:128 * (img + 1)]
            zt_y = midp2.tile([128, 2048], F32, tag="zt_y")
            for c4 in range(4):
                p = psp.tile([128, 512], F32, tag="ps")
                nc.tensor.matmul(out=p[:], lhsT=wimg_y,
                                 rhs=xt_y[:, 512 * c4:512 * (c4 + 1)],
                                 start=True, stop=True)
                nc.scalar.activation(out=zt_y[:, 512 * c4:512 * (c4 + 1)], in_=p[:], func=COPY)
            wimg_c = lhstyc_t[:, 128 * img:128 * (img + 1)]
            zt_c = midp2.tile([128, 1024], F32, tag="zt_c")
            for c2 in range(2):
                p = psp.tile([128, 512], F32, tag="ps")
                nc.tensor.matmul(out=p[:], lhsT=wimg_c,
                                 rhs=xt_c[:, 512 * c2:512 * (c2 + 1)],
                                 start=True, stop=True)
                nc.scalar.activation(out=zt_c[:, 512 * c2:512 * (c2 + 1)], in_=p[:], func=COPY)

            if STAGES < 3:
                dtile = outbp.tile([128, 512], F32, tag="r_t")
                nc.vector.tensor_copy(dtile[:], zt_y[:, 0:512])
                nc.sync.dma_start(out=out_d[img, 0, 0:128, :], in_=dtile[:])
                continue
            # ---------------- T2 (Y): per-ii picks -----------------------
            # B3'[part=(a,P0,jw), f3' = s*1024 + v*128 + ii*8 + x]
            b3_y = midp2.tile([128, 2048], F32, tag="b3_y")
            b3_y_w = b3_y[:].rearrange("k (s v ii x) -> k s v ii x", s=2, v=8, ii=16, x=8)
            for iq in range(4):
                p = psp.tile([128, 512], F32, tag="ps")
                for il in range(4):
                    ii = 4 * iq + il
                    nc.tensor.transpose(out=p[:, 128 * il:128 * (il + 1)],
                                        in_=zt_y[:, 128 * ii:128 * (ii + 1)],
                                        identity=ident[:])
                srcr = p[:].rearrange("k (il s x v) -> k il s x v", il=4, s=2, x=8)
                for s in range(2):
                    dst = b3_y_w[:, s, :, 4 * iq:4 * (iq + 1), :].rearrange(
                        "k v il x -> k il x v")
                    nc.scalar.activation(out=dst, in_=srcr[:, :, s], func=COPY)

            # ---------------- T2 (C): per-ii 64-picks --------------------
            # B3c'[part=(ch,a,P10,w3h) 64, f3c' = s*1024 + v*128 + ii*8 + x]
            b3_c = midp2.tile([64, 2048], F32, tag="b3_c")
            b3_c_w = b3_c[:].rearrange("k (s v ii x) -> k s v ii x", s=2, v=8, ii=16, x=8)
            for iq in range(4):
                p = psp.tile([128, 512], F32, tag="ps")
                for il in range(4):
                    ii = 4 * iq + il
                    nc.tensor.transpose(out=p[0:64, 128 * il:128 * (il + 1)],
                                        in_=zt_c[:, 64 * ii:64 * (ii + 1)],
                                        identity=ident[:])
                srcr = p[0:64, :].rearrange("k (il s x v) -> k il s x v", il=4, s=2, x=8)
                for s in range(2):
                    dst = b3_c_w[:, s, :, 4 * iq:4 * (iq + 1), :].rearrange(
                        "k v il x -> k il x v")
                    nc.scalar.activation(out=dst, in_=srcr[:, :, s], func=COPY)

            if STAGES < 4:
                dtile = outbp.tile([128, 512], F32, tag="r_t")
                nc.vector.tensor_copy(dtile[:], b3_y[:, 0:512])
                nc.sync.dma_start(out=out_d[img, 0, 0:128, :], in_=dtile[:])
                continue
            # ---------------- T3 (Y): picks (ii, x) ----------------------
            # B4[part=(ii,x), f4 = a*512 + P0*256 + jw*16 + s*8 + v]
            b4_y = midp.tile([128, 2048], MMDT, tag="b4_y")
            b4_y_r = b4_y[:].rearrange("k (ap jw s v) -> k ap jw s v", ap=8, jw=16, s=2, v=8)
            for s in range(2):
                for vq in range(2):
                    p = psp.tile([128, 512], F32, tag="ps")
                    for vj in range(4):
                        v = vq * 4 + vj
                        base = s * 1024 + v * 128
                        nc.tensor.transpose(out=p[:, 128 * vj:128 * (vj + 1)],
                                            in_=b3_y[:, base:base + 128],
                                            identity=ident[:])
                    src = p[:].rearrange("k (vj ap jw) -> k vj ap jw", vj=4, ap=8)
                    dst = b4_y_r[:, :, :, s, vq * 4:(vq + 1) * 4].rearrange(
                        "k ap jw vj -> k vj ap jw")
                    nc.vector.tensor_copy(dst, src)

            # ---------------- T3 (C): picks (ii', x) ---------------------
            # B4c[part=(ii',x), f4c = ch*512 + a*256 + P10*64 + w3h*16 + s*8 + v]
            b4_c = midp.tile([128, 1024], MMDT, tag="b4_c")
            b4_c2 = b4_c[:].rearrange("k (ca pw s v) -> k ca pw s v", ca=4, pw=16, s=2, v=8)
            for s in range(2):
                for vq in range(2):
                    p = psp.tile([128, 512], F32, tag="ps")
                    for vj in range(4):
                        v = vq * 4 + vj
                        base = s * 1024 + v * 128
                        nc.tensor.transpose(out=p[:, 64 * vj:64 * (vj + 1)],
                                            in_=b3_c[0:64, base:base + 128],
                                            identity=ident[0:64, 0:64])
                    src = p[:, 0:256].rearrange("k (vj ca pw) -> k vj ca pw", vj=4, ca=4)
                    dst = b4_c2[:, :, :, s, vq * 4:(vq + 1) * 4].rearrange(
                        "k ca pw vj -> k vj ca pw")
                    nc.vector.tensor_copy(dst, src)

            if STAGES < 5:
                dtile = outbp.tile([128, 512], F32, tag="r_t")
                nc.vector.tensor_copy(dtile[:], b4_y[:, 0:512].bitcast(F32))
                nc.sync.dma_start(out=out_d[img, 0, 0:128, :], in_=dtile[:])
                continue
            # ---------------- x-matmul (Y) + drain -----------------------
            y_t = midp.tile([128, 2048], F32, tag="y_t")
            for a in range(4):
                p = psp.tile([128, 512], F32, tag="ps")
                nc.tensor.matmul(out=p[:], lhsT=lhstx_t[:],
                                 rhs=b4_y[:, 512 * a:512 * (a + 1)],
                                 start=True, stop=True)
                sl = slice(512 * a, 512 * (a + 1))
                nc.scalar.activation(out=y_t[:, sl], in_=p[:], func=COPY)

            if STAGES < 6:
                dtile = outbp.tile([128, 512], F32, tag="r_t")
                nc.vector.tensor_copy(dtile[:], y_t[:, 0:512])
                nc.sync.dma_start(out=out_d[img, 0, 0:128, :], in_=dtile[:])
                continue
            # ---------------- fused chroma ups matmuls + col-ups ---------
            for b in range(4):
                mps = []
                for mi, (name, srcs) in enumerate(UPS_MAPS):
                    p = psp.tile([128, 512], F32, tag="ps")
                    calls = []
                    for ch, scale in srcs:
                        for wi, (wb, half) in enumerate(UPS_WINDOWS):
                            if wb != b:
                                continue
                            calls.append((mi, ch, wi, half))
                    for idx, (mi2, ch, wi, half) in enumerate(calls):
                        rhs = b4_c[:, 512 * ch + 256 * half: 512 * ch + 256 * (half + 1)]
                        tail(nc.tensor.matmul(out=p[:, 0:256], lhsT=upsw(mi2, ch, wi),
                                              rhs=rhs, start=(idx == 0),
                                              stop=(idx == len(calls) - 1)), img)
                    mps.append(p)

                ups_sb = []
                map_bias = (C_R, C_B, -C_G)
                for mi, p in enumerate(mps):
                    q3 = outbp.tile([128, 256], F32, tag="q3")
                    q1 = outbp.tile([128, 256], F32, tag="q1")
                    tail(nc.scalar.activation(out=q3[:], in_=p[:, 0:256], func=COPY,
                                              scale=0.75, bias=0.75 * map_bias[mi]), img)
                    tail(nc.scalar.activation(out=q1[:], in_=p[:, 0:256], func=COPY,
                                              scale=0.25, bias=0.25 * map_bias[mi]), img)
                    m_up = outbp.tile([128, 512], F32, tag=f"mup_{mi}")
                    m2 = m_up[:].rearrange("k (c two) -> k c two", two=2)
                    nc.vector.tensor_add(m2[:, 1:256, 0], q3[:, 1:256], q1[:, 0:255])
                    nc.vector.tensor_add(m2[:, 0:255, 1], q3[:, 0:255], q1[:, 1:256])
                    nc.vector.tensor_add(m_up[:, 0:1], q3[:, 0:1], q1[:, 0:1])
                    tail(nc.vector.tensor_add(m_up[:, 511:512], q3[:, 255:256], q1[:, 255:256]), img)
                    ups_sb.append(m_up)

                # ---------------- color combine + store ------------------
                sl = slice(512 * b, 512 * (b + 1))
                r_t = outbp.tile([128, 512], F32, tag="r_t")
                g_t = outbp.tile([128, 512], F32, tag="g_t")
                bl_t = outbp.tile([128, 512], F32, tag="bl_t")
                tail(nc.vector.tensor_add(r_t[:], y_t[:, sl], ups_sb[0][:]), img)
                tail(nc.vector.tensor_sub(g_t[:], y_t[:, sl], ups_sb[2][:]), img)
                tail(nc.vector.tensor_add(bl_t[:], y_t[:, sl], ups_sb[1][:]), img)
                rows = slice(128 * b, 128 * (b + 1))
                if STAGES >= 7 or b == 0:
                    tail(nc.sync.dma_start(out=out_d[img, 0, rows, :], in_=r_t[:]), img)
                if STAGES >= 7:
                    tail(nc.sync.dma_start(out=out_d[img, 1, rows, :], in_=g_t[:]), img)
                    tail(nc.sync.dma_start(out=out_d[img, 2, rows, :], in_=bl_t[:]), img)

        # tail absorb: make SP observe all pending ticks so the final Tile
        # drain needs <=2 sem waits (walrus CTRL-queue cap)
        for prod in tails:
            n = nc.sync.nop()
            add_dep_helper(n.ins, prod.ins, sync=True, reason="tail absorb")


# ------------------------------------------------------------------ entry

_NC_CACHE = {}


def kernel(input_y, input_cb, input_cr, jpeg_quality,
           quantization_table_y, quantization_table_c, H, W):
    input_y = np.ascontiguousarray(np.asarray(input_y), dtype=np.float32)
    input_cb = np.ascontiguousarray(np.asarray(input_cb), dtype=np.float32)
    input_cr = np.ascontiguousarray(np.asarray(input_cr), dtype=np.float32)
    q = np.asarray(jpeg_quality, dtype=np.float32)
    qt_y = np.asarray(quantization_table_y, dtype=np.float32).reshape(8, 8)
    qt_c = np.asarray(quantization_table_c, dtype=np.float32).reshape(8, 8)
    B = input_y.shape[0]
    assert int(H) == 512 and int(W) == 512 and B == 64

    if "nc" not in _NC_CACHE:
        _NC_CACHE["nc"] = build_nc()
    nc = _NC_CACHE["nc"]

    n_cores = 8
    in_maps = _prep_in_maps(input_y, input_cb, input_cr, q, qt_y, qt_c, n_cores)
    res = run_bass_kernel_spmd(nc, in_maps, list(range(n_cores)))
    out = np.concatenate([res.results[c]["rgb"][0] for c in range(n_cores)], axis=0)
    return out.astype(np.float32)


def _prep_in_maps(input_y, input_cb, input_cr, q, qt_y, qt_c, n_cores=8):
    lhsty, lhstyc, lhstx, ups = host_consts(q, qt_y, qt_c)
    B = input_y.shape[0]
    per = B // n_cores
    in_maps = []
    for c in range(n_cores):
        sl = slice(c * per, (c + 1) * per)
        in_maps.append({
            "wy": input_y[sl].reshape(per, 4096, 64),
            "wcb": input_cb[sl].reshape(per, 1024, 64),
            "wcr": input_cr[sl].reshape(per, 1024, 64),
            "lhsty": lhsty[sl],
            "lhstyc": lhstyc[sl],
            "lhstx": lhstx,
            "upsw": ups,
        })
    return in_maps


def _make_sharded(nc, in_maps):
    import jax
    from jax.sharding import Mesh, PartitionSpec
    from jax.experimental.shard_map import shard_map
    from concourse import bass2jax, mybir as mb

    n_cores = len(in_maps)
    partition_name = nc.partition_id_tensor.name if nc.partition_id_tensor else None
    in_names, out_names, out_avals, zero_outs = [], [], [], []
    for alloc in nc.m.functions[0].allocations:
        if not isinstance(alloc, mb.MemoryLocationSet):
            continue
        name = alloc.memorylocations[0].name
        if alloc.kind == "ExternalInput":
            if name != partition_name:
                in_names.append(name)
        elif alloc.kind == "ExternalOutput":
            shape = tuple(alloc.tensor_shape)
            dtype = mb.dt.np(alloc.dtype)
            out_names.append(name)
            out_avals.append(jax.core.ShapedArray(shape, dtype))
            zero_outs.append(np.zeros(shape, dtype))
    n_params = len(in_names)
    all_in = in_names + out_names + ([partition_name] if partition_name else [])

    def _body(*args):
        operands = list(args)
        if partition_name is not None:
            operands.append(bass2jax.partition_id_tensor())
        outs = bass2jax._bass_exec_p.bind(
            *operands, out_avals=tuple(out_avals), in_names=tuple(all_in),
            out_names=tuple(out_names), lowering_input_output_aliases=(),
            sim_require_finite=True, sim_require_nnan=True, nc=nc)
        return tuple(outs)

    devices = jax.devices()[:n_cores]
    mesh = Mesh(np.asarray(devices), ("core",))
    nin = n_params + len(out_names)
    sharded = jax.jit(
        shard_map(_body, mesh=mesh, in_specs=(PartitionSpec("core"),) * nin,
                  out_specs=(PartitionSpec("core"),) * len(out_names),
                  check_rep=False),
        keep_unused=True)
    concat_in = [np.concatenate([np.asarray(in_maps[c][nm]) for c in range(n_cores)], axis=0)
                 for nm in in_names]
    concat_zero = [np.zeros((n_cores * z.shape[0], *z.shape[1:]), z.dtype) for z in zero_outs]
    dev_in = [jax.device_put(a) for a in concat_in + concat_zero]
    return sharded, dev_in


def time_kernel(inputs, reps=16, program_reps=5):
    """Estimate per-batch (64-image) exec ns via repeat-program differencing:
    exec = (T(program_reps) - T(1)) / (program_reps - 1); RPC overheads cancel."""
    import jax
    import time as _t
    from concourse import bass2jax

    bass2jax.install_neuronx_cc_hook()
    input_y = np.ascontiguousarray(np.asarray(inputs["input_y"]), dtype=np.float32)
    input_cb = np.ascontiguousarray(np.asarray(inputs["input_cb"]), dtype=np.float32)
    input_cr = np.ascontiguousarray(np.asarray(inputs["input_cr"]), dtype=np.float32)
    q = np.asarray(inputs["jpeg_quality"], dtype=np.float32)
    qt_y = np.asarray(inputs["quantization_table_y"], dtype=np.float32).reshape(8, 8)
    qt_c = np.asarray(inputs["quantization_table_c"], dtype=np.float32).reshape(8, 8)
    in_maps = _prep_in_maps(input_y, input_cb, input_cr, q, qt_y, qt_c)

    def bench(prog_reps):
        key = f"nc{prog_reps}"
        if key not in _NC_CACHE:
            _NC_CACHE[key] = build_nc(reps=prog_reps)
        sharded, dev_in = _make_sharded(_NC_CACHE[key], in_maps)
        jax.block_until_ready(sharded(*dev_in))  # warm
        times = []
        for _ in range(reps):
            t0 = _t.time()
            jax.block_until_ready(sharded(*dev_in))
            times.append(_t.time() - t0)
        return min(times), sorted(times)[len(times) // 2]

    t1_min, t1_med = bench(1)
    tR_min, tR_med = bench(program_reps)
    per_min = (tR_min - t1_min) / (program_reps - 1)
    per_med = (tR_med - t1_med) / (program_reps - 1)
    print(f"  T(1) min/med: {t1_min*1e3:.2f}/{t1_med*1e3:.2f} ms; "
          f"T({program_reps}) min/med: {tR_min*1e3:.2f}/{tR_med*1e3:.2f} ms")
    print(f"  per-batch exec: min-diff {per_min*1e6:.1f} us, med-diff {per_med*1e6:.1f} us")
    return per_med * 1e9


if __name__ == "__main__":
    rng = np.random.default_rng(0)
    B = 64
    inputs = dict(
        input_y=(rng.standard_normal((B, 4096, 8, 8)) * 10).astype(np.float32),
        input_cb=(rng.standard_normal((B, 1024, 8, 8)) * 10).astype(np.float32),
        input_cr=(rng.standard_normal((B, 1024, 8, 8)) * 10).astype(np.float32),
        jpeg_quality=rng.uniform(10, 95, size=B).astype(np.float32),
        quantization_table_y=QT_Y[None],
        quantization_table_c=QT_C[None],
        H=512, W=512,
    )
    out = kernel(**inputs)
    print("out", out.shape, out.dtype, float(np.abs(out).max()))



# revision 5
# speedup vs baseline: 716.2154x; 57.0764x over previous
"""DiffJPEG decode kernel for Trainium2 (8 NeuronCores, batch-parallel).

Pipeline per image (validated in numpy against the reference, see sim.py):
  Y:  natural DMA load -> T_in (PE transpose) -> y-matmul (dequant+col-IDCT
      folded into per-image lhsT) -> T2 -> T3 (PE transposes that convert the
      block layout to image-row layout) -> x-matmul (row-IDCT) -> biased
      drains (color constants folded)
  C:  same front; then fused row-upsample+row-IDCT matmuls (color scales
      1.403/1.773/0.344/0.714 folded into constant lhsTs), col-upsample on
      DVE via shifted adds, color combine on DVE.

Layout bit-conventions (Y, n in [0,4096)):
  n = 256 t + 2 p + s   (t:16, p:128 partitions, s:2)
  unpatchify: a = t[3:2], ii = (t[1:0], p[6:5]), j = (p[4:0], s)
  row r = 128 a + 8 ii + u,  col c = 16 p[4:0] + 8 s + v
Chroma (n' in [0,1024)): n' = 256 t' + 2 p + s; a' = t'[1],
  ii' = (t'[0], p[6:4]), j' = (p[3:0], s).
"""
import os
import sys
import numpy as np

sys.path.insert(0, "/opt/trn_rl_repo")

import concourse.bass as bass
import concourse.mybir as mybir
import concourse.tile as tile
from concourse.tile import add_dep_helper
from concourse.bass_utils import run_bass_kernel_spmd
from concourse.masks import make_identity

F32 = mybir.dt.float32
F32R = mybir.dt.float32r
COPY = mybir.ActivationFunctionType.Copy

# ------------------------------------------------------------------ host math

QT_Y = np.array([[16,11,10,16,24,40,51,61],[12,12,14,19,26,58,60,55],[14,13,16,24,40,57,69,56],[14,17,22,29,51,87,80,62],[18,22,37,56,68,109,103,77],[24,35,55,64,81,104,113,92],[49,64,78,87,103,121,120,101],[72,92,95,98,112,100,103,99]], dtype=np.float32)
QT_C = np.array([[17,18,24,47,99,99,99,99],[18,21,26,66,99,99,99,99],[24,26,56,99,99,99,99,99],[47,66,99,99,99,99,99,99],[99,99,99,99,99,99,99,99],[99,99,99,99,99,99,99,99],[99,99,99,99,99,99,99,99],[99,99,99,99,99,99,99,99]], dtype=np.float32)

SCALE_CR2 = np.float32(1.403)
SCALE_CB2 = np.float32(1.773)
SCALE_GC_CB = np.float32(0.344)
SCALE_GC_CR = np.float32(0.714)
_K = np.float32(128.0 / 255.0)
_OFF = np.float32(128.0 / 255.0 - 0.5)
C_R = float(_K + SCALE_CR2 * _OFF)
C_G = float(_K - (SCALE_GC_CB + SCALE_GC_CR) * _OFF)
C_B = float(_K + SCALE_CB2 * _OFF)

# (b, half) windows with nonzero fused-upsample weight
UPS_WINDOWS = [(0, 0), (1, 0), (1, 1), (2, 0), (2, 1), (3, 1)]
# map sources: (name, [(channel, scale), ...]); channel 0=cb, 1=cr
UPS_MAPS = [("cr2", [(1, SCALE_CR2)]),
            ("cb2", [(0, SCALE_CB2)]),
            ("gc", [(0, SCALE_GC_CB), (1, SCALE_GC_CR)])]


def _poly_floor_np(x):
    f = np.floor(x)
    return (f + (x - np.float32(0.5) - f) ** 3).astype(np.float32)


def _diff_clip_np(x, mn, mx, scale=np.float32(0.02)):
    with np.errstate(over="ignore"):
        x = np.where(x > mx, -scale * (np.exp(-x + mx) - np.float32(1.0)) + mx, x)
        x = np.where(x < mn, scale * (np.exp(x - mn) - np.float32(1.0)) + mn, x)
    return x.astype(np.float32)


def dequant_factor(q, qt):
    q = q.astype(np.float32)
    s = _poly_floor_np(np.where(q < 50.0, np.float32(5000.0) / q, np.float32(200.0) - 2.0 * q))
    qts = qt[None, :, :] * s[:, None, None]
    return _poly_floor_np(_diff_clip_np((qts + np.float32(50.0)) / np.float32(100.0), np.float32(1.0), np.float32(255.0)))


def idct_A():
    x = np.arange(8, dtype=np.float64)
    u = np.arange(8, dtype=np.float64)
    alpha = np.ones(8, dtype=np.float64)
    alpha[0] = 1.0 / np.sqrt(2.0)
    A = 0.5 * alpha[:, None] * np.cos((2.0 * u[None, :] + 1.0) * x[:, None] * np.pi / 16.0)
    return A.astype(np.float32)


def upsample_U(n_in):
    n_out = 2 * n_in
    U = np.zeros((n_out, n_in), dtype=np.float32)
    for R in range(n_out):
        k, odd = divmod(R, 2)
        if odd:
            U[R, k] += 0.75
            U[R, min(k + 1, n_in - 1)] += 0.25
        else:
            U[R, k] += 0.75
            U[R, max(k - 1, 0)] += 0.25
    return U


def make_lhsT_y(F):
    """[128,128]: k=(s,x,y)->m=(s,x,v): F[x,y]*A[y,v]/255 (diag in s,x)."""
    A = idct_A()
    W = np.zeros((2, 8, 8, 2, 8, 8), dtype=np.float32)
    for s in range(2):
        for xx in range(8):
            W[s, xx, :, s, xx, :] = (F[xx, :, None] * A) / np.float32(255.0)
    return W.reshape(128, 128)


def make_lhsT_x():
    """[128,128]: k=(ii,x)->m=(ii,u): A[x,u] (diag in ii)."""
    A = idct_A()
    W = np.zeros((16, 8, 16, 8), dtype=np.float32)
    for ii in range(16):
        W[ii, :, ii, :] = A
    return W.reshape(128, 128)


def make_ups_lhsT(b, half, scale):
    """[128,128] fused row-upsample+row-IDCT for chroma, scaled."""
    A = idct_A()
    U = upsample_U(256)
    W = np.zeros((16, 8, 128), dtype=np.float32)
    for ii in range(16):
        ip = 16 * half + ii
        Ublk = U[128 * b:128 * (b + 1), 8 * ip:8 * ip + 8]
        W[ii] = np.float32(scale) * (A @ Ublk.T)
    return W.reshape(128, 128)


def host_consts(jpeg_quality, qt_y, qt_c):
    B = jpeg_quality.shape[0]
    Fy = dequant_factor(jpeg_quality, qt_y)
    Fc = dequant_factor(jpeg_quality, qt_c)
    lhsty = np.stack([make_lhsT_y(Fy[i]) for i in range(B)])
    lhstyc = np.stack([make_lhsT_y(Fc[i]) for i in range(B)])
    lhstx = make_lhsT_x()
    ups = np.zeros((3, 2, 6, 128, 128), dtype=np.float32)
    for mi, (name, srcs) in enumerate(UPS_MAPS):
        for ch, scale in srcs:
            for wi, (b, half) in enumerate(UPS_WINDOWS):
                ups[mi, ch, wi] = make_ups_lhsT(b, half, scale)
    return lhsty, lhstyc, lhstx, ups


# ------------------------------------------------------------------ device

def split_excess_waits(nc, max_waits=1):
    """Walrus caps sem-waits per instruction; hoist excess onto same-engine
    NOPs inserted immediately before (same sequencer => semantics equal)."""
    for f in nc.m.functions:
        for blk in f.blocks:
            insts = blk.instructions
            idx = 0
            while idx < len(insts):
                inst = insts[idx]
                si = inst.sync_info
                if si is not None and si.on_wait is not None and len(si.on_wait) > max_waits:
                    waits = list(si.on_wait)
                    keep = waits[-max_waits:]
                    excess = waits[:-max_waits]
                    pos = idx
                    for c0 in range(0, len(excess), max_waits):
                        chunk = excess[c0:c0 + max_waits]
                        nop = mybir.InstNoOp(name=nc.get_next_instruction_name(),
                                             engine=inst.engine, ins=[], outs=[])
                        nop.sync_info = mybir.SyncInfo(on_wait=chunk, on_update=[])
                        nc.register_instruction(nop)
                        insts.insert(pos, nop)
                        pos += 1
                        idx += 1
                    si.on_wait = keep
                idx += 1


IMGS_PER_CORE = 8
STAGES = int(os.environ.get("KERNEL_STAGES", "7"))
ALIAS_OUT = bool(int(os.environ.get("KERNEL_ALIAS_OUT", "0")))
USE_F32R = True  # dtype for ymm/xmm/ups matmul inputs (transposes stay f32)
MMDT = F32R if USE_F32R else F32


def build_nc(reps=1):
    nc = bass.Bass()
    I = IMGS_PER_CORE

    wy_d = nc.dram_tensor("wy", [I, 4096, 64], F32, kind="ExternalInput")
    wcb_d = nc.dram_tensor("wcb", [I, 1024, 64], F32, kind="ExternalInput")
    wcr_d = nc.dram_tensor("wcr", [I, 1024, 64], F32, kind="ExternalInput")
    lhsty_d = nc.dram_tensor("lhsty", [I, 128, 128], F32, kind="ExternalInput")
    lhstyc_d = nc.dram_tensor("lhstyc", [I, 128, 128], F32, kind="ExternalInput")
    lhstx_d = nc.dram_tensor("lhstx", [128, 128], F32, kind="ExternalInput")
    ups_d = nc.dram_tensor("upsw", [3, 2, 6, 128, 128], F32, kind="ExternalInput")
    nout = 1 if ALIAS_OUT else reps
    out_d = nc.dram_tensor("rgb", [nout, I, 3, 512, 512], F32, kind="ExternalOutput")

    with tile.TileContext(nc) as tc:
        for rep in range(reps):
            _build_body(nc, tc, wy_d, wcb_d, wcr_d, lhsty_d, lhstyc_d, lhstx_d,
                        ups_d, out_d[0 if ALIAS_OUT else rep])
    split_excess_waits(nc)
    return nc


def _build_body(nc, tc, wy_d, wcb_d, wcr_d, lhsty_d, lhstyc_d, lhstx_d, ups_d, out_d):
    I = IMGS_PER_CORE
    tails = []

    def tail(inst, img):
        if img == I - 1:
            tails.append(inst)
        return inst

    with tc.tile_pool(name="const", bufs=1) as constp, \
         tc.tile_pool(name="ld", bufs=2) as ldp, \
         tc.tile_pool(name="mid", bufs=1) as midp, \
         tc.tile_pool(name="mid2", bufs=2) as midp2, \
         tc.tile_pool(name="outb", bufs=2) as outbp, \
         tc.tile_pool(name="ps", bufs=8, space="PSUM") as psp:

        ident = constp.tile([128, 128], F32, tag="ident")
        make_identity(nc, ident[:])

        # constant weights: DMA f32 then ACT-recast to matmul dtype (walrus
        # requires fp32r matmul inputs to be produced by a rounding op)
        lhstx_f = constp.tile([128, 128], F32, tag="lhstx_f")
        nc.sync.dma_start(out=lhstx_f[:], in_=lhstx_d[:])
        lhstx_t = constp.tile([128, 128], MMDT, tag="lhstx")
        nc.scalar.activation(out=lhstx_t[:], in_=lhstx_f[:], func=COPY)

        lhsty_f = constp.tile([128, 128 * I], F32, tag="lhsty_f")
        nc.sync.dma_start(out=lhsty_f[:].rearrange("k (i m) -> k i m", i=I),
                          in_=lhsty_d.rearrange("i k m -> k i m"))
        lhsty_t = constp.tile([128, 128 * I], MMDT, tag="lhsty")
        nc.scalar.activation(out=lhsty_t[:], in_=lhsty_f[:], func=COPY)

        lhstyc_f = constp.tile([128, 128 * I], F32, tag="lhstyc_f")
        nc.sync.dma_start(out=lhstyc_f[:].rearrange("k (i m) -> k i m", i=I),
                          in_=lhstyc_d.rearrange("i k m -> k i m"))
        lhstyc_t = constp.tile([128, 128 * I], MMDT, tag="lhstyc")
        nc.scalar.activation(out=lhstyc_t[:], in_=lhstyc_f[:], func=COPY)

        upsw_f = constp.tile([128, 3 * 2 * 6 * 128], F32, tag="upsw_f")
        nc.sync.dma_start(out=upsw_f[:].rearrange("k (m c w n) -> k m c w n", m=3, c=2, w=6),
                          in_=ups_d.rearrange("m c w k n -> k m c w n"))
        upsw_t = constp.tile([128, 3 * 2 * 6 * 128], MMDT, tag="upsw")
        nc.scalar.activation(out=upsw_t[:], in_=upsw_f[:], func=COPY)

        def upsw(mi, ch, wi):
            off = ((mi * 2 + ch) * 6 + wi) * 128
            return upsw_t[:, off:off + 128]

        for img in range(I):
            # ---------------- loads (8KB / 2KB contiguous runs) ----------
            xnat_y = ldp.tile([128, 2048], F32, tag="xnat_y")
            nc.sync.dma_start(out=xnat_y[:],
                              in_=wy_d[img].rearrange("(p w) c -> p (w c)", p=128, w=32))
            xnat_c = ldp.tile([128, 1024], F32, tag="xnat_c")
            nc.sync.dma_start(out=xnat_c[:, 0:512],
                              in_=wcb_d[img].rearrange("(p w) c -> p (w c)", p=128, w=8))
            nc.sync.dma_start(out=xnat_c[:, 512:1024],
                              in_=wcr_d[img].rearrange("(p w) c -> p (w c)", p=128, w=8))

            if STAGES < 1:
                dtile = outbp.tile([128, 512], F32, tag="r_t")
                nc.vector.tensor_copy(dtile[:], xnat_y[:, 0:512])
                nc.sync.dma_start(out=out_d[img, 0, 0:128, :], in_=dtile[:])
                continue
            # ---------------- T_in (Y): per-jw picks ---------------------
            # XT'[k=(s,x,y), f1 = ii*128 + a*32 + P0*16 + jw]
            xt_y = midp2.tile([128, 2048], MMDT, tag="xt_y")
            xt_y_r = xt_y[:].rearrange("k (ii a P0 jw) -> k ii a P0 jw",
                                       ii=16, a=4, P0=2, jw=16)
            for jq in range(4):
                p = psp.tile([128, 512], F32, tag="ps")
                for jl in range(4):
                    jw = 4 * jq + jl
                    nc.tensor.transpose(out=p[:, 128 * jl:128 * (jl + 1)],
                                        in_=xnat_y[:, 128 * jw:128 * (jw + 1)],
                                        identity=ident[:])
                srcr = p[:].rearrange("k (jl a ii P0) -> k jl a ii P0", jl=4, a=4, ii=16)
                for a in range(4):
                    dst = xt_y_r[:, :, a, :, 4 * jq:4 * (jq + 1)].rearrange(
                        "k ii P0 jl -> k jl ii P0")
                    nc.scalar.activation(out=dst, in_=srcr[:, :, a], func=COPY)

            # ---------------- T_in (C): per (ch, w3h) picks --------------
            # XT_c'[k, f1c = ii*64 + ch*32 + a*16 + P10*4 + w3h]
            xt_c = midp2.tile([128, 1024], MMDT, tag="xt_c")
            xt_c_r = xt_c[:].rearrange("k (ii ch a P10 w3h) -> k ii ch a P10 w3h",
                                       ii=16, ch=2, a=2, P10=4, w3h=4)
            for ch in range(2):
                p = psp.tile([128, 512], F32, tag="ps")
                for w3h in range(4):
                    base = 512 * ch + 128 * w3h
                    nc.tensor.transpose(out=p[:, 128 * w3h:128 * (w3h + 1)],
                                        in_=xnat_c[:, base:base + 128],
                                        identity=ident[:])
                srcr = p[:].rearrange("k (w3h a ii P10) -> k w3h a ii P10",
                                      w3h=4, a=2, ii=16)
                for a in range(2):
                    dst = xt_c_r[:, :, ch, a, :, :].rearrange(
                        "k ii P10 w3h -> k w3h ii P10")
                    nc.scalar.activation(out=dst, in_=srcr[:, :, a], func=COPY)

            if STAGES < 2:
                dtile = outbp.tile([128, 512], F32, tag="r_t")
                nc.vector.tensor_copy(dtile[:], xt_y[:, 0:512].bitcast(F32))
                nc.sync.dma_start(out=out_d[img, 0, 0:128, :], in_=dtile[:])
                continue
            # ---------------- y-matmul (contiguous drain) ----------------
            wimg_y = lhsty_t[:, 128 * img:128 * (img + 1)]
            zt_y = midp2.tile([128, 2048], F32, tag="zt_y")
            for c4 in range(4):
                p = psp.tile([128, 512], F32, tag="ps")
                nc.tensor.matmul(out=p[:], lhsT=wimg_y,
                                 rhs=xt_y[:, 512 * c4:512 * (c4 + 1)],
                                 start=True, stop=True)
                nc.scalar.activation(out=zt_y[:, 512 * c4:512 * (c4 + 1)], in_=p[:], func=COPY)
            wimg_c = lhstyc_t[:, 128 * img:128 * (img + 1)]
            zt_c = midp2.tile([128, 1024], F32, tag="zt_c")
            for c2 in range(2):
                p = psp.tile([128, 512], F32, tag="ps")
                nc.tensor.matmul(out=p[:], lhsT=wimg_c,
                                 rhs=xt_c[:, 512 * c2:512 * (c2 + 1)],
                                 start=True, stop=True)
                nc.scalar.activation(out=zt_c[:, 512 * c2:512 * (c2 + 1)], in_=p[:], func=COPY)

            if STAGES < 3:
                dtile = outbp.tile([128, 512], F32, tag="r_t")
                nc.vector.tensor_copy(dtile[:], zt_y[:, 0:512])
                nc.sync.dma_start(out=out_d[img, 0, 0:128, :], in_=dtile[:])
                continue
            # ---------------- T2 (Y): per-ii picks -----------------------
            # B3'[part=(a,P0,jw), f3' = s*1024 + v*128 + ii*8 + x]
            b3_y = midp2.tile([128, 2048], F32, tag="b3_y")
            b3_y_w = b3_y[:].rearrange("k (s v ii x) -> k s v ii x", s=2, v=8, ii=16, x=8)
            for iq in range(4):
                p = psp.tile([128, 512], F32, tag="ps")
                for il in range(4):
                    ii = 4 * iq + il
                    nc.tensor.transpose(out=p[:, 128 * il:128 * (il + 1)],
                                        in_=zt_y[:, 128 * ii:128 * (ii + 1)],
                                        identity=ident[:])
                srcr = p[:].rearrange("k (il s x v) -> k il s x v", il=4, s=2, x=8)
                for s in range(2):
                    dst = b3_y_w[:, s, :, 4 * iq:4 * (iq + 1), :].rearrange(
                        "k v il x -> k il x v")
                    nc.scalar.activation(out=dst, in_=srcr[:, :, s], func=COPY)

            # ---------------- T2 (C): per-ii 64-picks --------------------
            # B3c'[part=(ch,a,P10,w3h) 64, f3c' = s*1024 + v*128 + ii*8 + x]
            b3_c = midp2.tile([64, 2048], F32, tag="b3_c")
            b3_c_w = b3_c[:].rearrange("k (s v ii x) -> k s v ii x", s=2, v=8, ii=16, x=8)
            for iq in range(4):
                p = psp.tile([128, 512], F32, tag="ps")
                for il in range(4):
                    ii = 4 * iq + il
                    nc.tensor.transpose(out=p[0:64, 128 * il:128 * (il + 1)],
                                        in_=zt_c[:, 64 * ii:64 * (ii + 1)],
                                        identity=ident[:])
                srcr = p[0:64, :].rearrange("k (il s x v) -> k il s x v", il=4, s=2, x=8)
                for s in range(2):
                    dst = b3_c_w[:, s, :, 4 * iq:4 * (iq + 1), :].rearrange(
                        "k v il x -> k il x v")
                    nc.scalar.activation(out=dst, in_=srcr[:, :, s], func=COPY)

            if STAGES < 4:
                dtile = outbp.tile([128, 512], F32, tag="r_t")
                nc.vector.tensor_copy(dtile[:], b3_y[:, 0:512])
                nc.sync.dma_start(out=out_d[img, 0, 0:128, :], in_=dtile[:])
                continue
            # ---------------- T3 (Y): picks (ii, x) ----------------------
            # B4[part=(ii,x), f4 = a*512 + P0*256 + jw*16 + s*8 + v]
            b4_y = midp.tile([128, 2048], MMDT, tag="b4_y")
            b4_y_r = b4_y[:].rearrange("k (ap jw s v) -> k ap jw s v", ap=8, jw=16, s=2, v=8)
            for s in range(2):
                for vq in range(2):
                    p = psp.tile([128, 512], F32, tag="ps")
                    for vj in range(4):
                        v = vq * 4 + vj
                        base = s * 1024 + v * 128
                        nc.tensor.transpose(out=p[:, 128 * vj:128 * (vj + 1)],
                                            in_=b3_y[:, base:base + 128],
                                            identity=ident[:])
                    src = p[:].rearrange("k (vj ap jw) -> k vj ap jw", vj=4, ap=8)
                    dst = b4_y_r[:, :, :, s, vq * 4:(vq + 1) * 4].rearrange(
                        "k ap jw vj -> k vj ap jw")
                    nc.vector.tensor_copy(dst, src)

            # ---------------- T3 (C): picks (ii', x) ---------------------
            # B4c[part=(ii',x), f4c = ch*512 + a*256 + P10*64 + w3h*16 + s*8 + v]
            b4_c = midp.tile([128, 1024], MMDT, tag="b4_c")
            b4_c2 = b4_c[:].rearrange("k (ca pw s v) -> k ca pw s v", ca=4, pw=16, s=2, v=8)
            for s in range(2):
                for vq in range(2):
                    p = psp.tile([128, 512], F32, tag="ps")
                    for vj in range(4):
                        v = vq * 4 + vj
                        base = s * 1024 + v * 128
                        nc.tensor.transpose(out=p[:, 64 * vj:64 * (vj + 1)],
                                            in_=b3_c[0:64, base:base + 128],
                                            identity=ident[0:64, 0:64])
                    src = p[:, 0:256].rearrange("k (vj ca pw) -> k vj ca pw", vj=4, ca=4)
                    dst = b4_c2[:, :, :, s, vq * 4:(vq + 1) * 4].rearrange(
                        "k ca pw vj -> k vj ca pw")
                    nc.vector.tensor_copy(dst, src)

            if STAGES < 5:
                dtile = outbp.tile([128, 512], F32, tag="r_t")
                nc.vector.tensor_copy(dtile[:], b4_y[:, 0:512].bitcast(F32))
                nc.sync.dma_start(out=out_d[img, 0, 0:128, :], in_=dtile[:])
                continue
            # ---------------- x-matmul (Y) + drain -----------------------
            y_t = midp.tile([128, 2048], F32, tag="y_t")
            for a in range(4):
                p = psp.tile([128, 512], F32, tag="ps")
                nc.tensor.matmul(out=p[:], lhsT=lhstx_t[:],
                                 rhs=b4_y[:, 512 * a:512 * (a + 1)],
                                 start=True, stop=True)
                sl = slice(512 * a, 512 * (a + 1))
                nc.scalar.activation(out=y_t[:, sl], in_=p[:], func=COPY)

            if STAGES < 6:
                dtile = outbp.tile([128, 512], F32, tag="r_t")
                nc.vector.tensor_copy(dtile[:], y_t[:, 0:512])
                nc.sync.dma_start(out=out_d[img, 0, 0:128, :], in_=dtile[:])
                continue
            # ---------------- fused chroma ups matmuls + col-ups ---------
            for b in range(4):
                mps = []
                for mi, (name, srcs) in enumerate(UPS_MAPS):
                    p = psp.tile([128, 512], F32, tag="ps")
                    calls = []
                    for ch, scale in srcs:
                        for wi, (wb, half) in enumerate(UPS_WINDOWS):
                            if wb != b:
                                continue
                            calls.append((mi, ch, wi, half))
                    for idx, (mi2, ch, wi, half) in enumerate(calls):
                        rhs = b4_c[:, 512 * ch + 256 * half: 512 * ch + 256 * (half + 1)]
                        tail(nc.tensor.matmul(out=p[:, 0:256], lhsT=upsw(mi2, ch, wi),
                                              rhs=rhs, start=(idx == 0),
                                              stop=(idx == len(calls) - 1)), img)
                    mps.append(p)

                ups_sb = []
                map_bias = (C_R, C_B, -C_G)
                for mi, p in enumerate(mps):
                    q3 = outbp.tile([128, 256], F32, tag="q3")
                    q1 = outbp.tile([128, 256], F32, tag="q1")
                    tail(nc.scalar.activation(out=q3[:], in_=p[:, 0:256], func=COPY,
                                              scale=0.75, bias=0.75 * map_bias[mi]), img)
                    tail(nc.scalar.activation(out=q1[:], in_=p[:, 0:256], func=COPY,
                                              scale=0.25, bias=0.25 * map_bias[mi]), img)
                    m_up = outbp.tile([128, 512], F32, tag=f"mup_{mi}")
                    m2 = m_up[:].rearrange("k (c two) -> k c two", two=2)
                    nc.vector.tensor_add(m2[:, 1:256, 0], q3[:, 1:256], q1[:, 0:255])
                    nc.vector.tensor_add(m2[:, 0:255, 1], q3[:, 0:255], q1[:, 1:256])
                    nc.vector.tensor_add(m_up[:, 0:1], q3[:, 0:1], q1[:, 0:1])
                    tail(nc.vector.tensor_add(m_up[:, 511:512], q3[:, 255:256], q1[:, 255:256]), img)
                    ups_sb.append(m_up)

                # ---------------- color combine + store ------------------
                sl = slice(512 * b, 512 * (b + 1))
                r_t = outbp.tile([128, 512], F32, tag="r_t")
                g_t = outbp.tile([128, 512], F32, tag="g_t")
                bl_t = outbp.tile([128, 512], F32, tag="bl_t")
                tail(nc.vector.tensor_add(r_t[:], y_t[:, sl], ups_sb[0][:]), img)
                tail(nc.vector.tensor_sub(g_t[:], y_t[:, sl], ups_sb[2][:]), img)
                tail(nc.vector.tensor_add(bl_t[:], y_t[:, sl], ups_sb[1][:]), img)
                rows = slice(128 * b, 128 * (b + 1))
                if STAGES >= 7 or b == 0:
                    tail(nc.sync.dma_start(out=out_d[img, 0, rows, :], in_=r_t[:]), img)
                if STAGES >= 7:
                    tail(nc.sync.dma_start(out=out_d[img, 1, rows, :], in_=g_t[:]), img)
                    tail(nc.sync.dma_start(out=out_d[img, 2, rows, :], in_=bl_t[:]), img)

        # tail absorb: make SP observe all pending ticks so the final Tile
        # drain needs <=2 sem waits (walrus CTRL-queue cap)
        for prod in tails:
            n = nc.sync.nop()
            add_dep_helper(n.ins, prod.ins, sync=True, reason="tail absorb")


# ------------------------------------------------------------------ entry

_NC_CACHE = {}


def kernel(input_y, input_cb, input_cr, jpeg_quality,
           quantization_table_y, quantization_table_c, H, W):
    input_y = np.ascontiguousarray(np.asarray(input_y), dtype=np.float32)
    input_cb = np.ascontiguousarray(np.asarray(input_cb), dtype=np.float32)
    input_cr = np.ascontiguousarray(np.asarray(input_cr), dtype=np.float32)
    q = np.asarray(jpeg_quality, dtype=np.float32)
    qt_y = np.asarray(quantization_table_y, dtype=np.float32).reshape(8, 8)
    qt_c = np.asarray(quantization_table_c, dtype=np.float32).reshape(8, 8)
    B = input_y.shape[0]
    assert int(H) == 512 and int(W) == 512 and B == 64

    if "nc" not in _NC_CACHE:
        _NC_CACHE["nc"] = build_nc()
    nc = _NC_CACHE["nc"]

    n_cores = 8
    in_maps = _prep_in_maps(input_y, input_cb, input_cr, q, qt_y, qt_c, n_cores)
    res = run_bass_kernel_spmd(nc, in_maps, list(range(n_cores)))
    out = np.concatenate([res.results[c]["rgb"][0] for c in range(n_cores)], axis=0)
    return out.astype(np.float32)


def _prep_in_maps(input_y, input_cb, input_cr, q, qt_y, qt_c, n_cores=8):
    lhsty, lhstyc, lhstx, ups = host_consts(q, qt_y, qt_c)
    B = input_y.shape[0]
    per = B // n_cores
    in_maps = []
    for c in range(n_cores):
        sl = slice(c * per, (c + 1) * per)
        in_maps.append({
            "wy": input_y[sl].reshape(per, 4096, 64),
            "wcb": input_cb[sl].reshape(per, 1024, 64),
            "wcr": input_cr[sl].reshape(per, 1024, 64),
            "lhsty": lhsty[sl],
            "lhstyc": lhstyc[sl],
            "lhstx": lhstx,
            "upsw": ups,
        })
    return in_maps


def _make_sharded(nc, in_maps):
    import jax
    from jax.sharding import Mesh, PartitionSpec
    from jax.experimental.shard_map import shard_map
    from concourse import bass2jax, mybir as mb

    n_cores = len(in_maps)
    partition_name = nc.partition_id_tensor.name if nc.partition_id_tensor else None
    in_names, out_names, out_avals, zero_outs = [], [], [], []
    for alloc in nc.m.functions[0].allocations:
        if not isinstance(alloc, mb.MemoryLocationSet):
            continue
        name = alloc.memorylocations[0].name
        if alloc.kind == "ExternalInput":
            if name != partition_name:
                in_names.append(name)
        elif alloc.kind == "ExternalOutput":
            shape = tuple(alloc.tensor_shape)
            dtype = mb.dt.np(alloc.dtype)
            out_names.append(name)
            out_avals.append(jax.core.ShapedArray(shape, dtype))
            zero_outs.append(np.zeros(shape, dtype))
    n_params = len(in_names)
    all_in = in_names + out_names + ([partition_name] if partition_name else [])

    def _body(*args):
        operands = list(args)
        if partition_name is not None:
            operands.append(bass2jax.partition_id_tensor())
        outs = bass2jax._bass_exec_p.bind(
            *operands, out_avals=tuple(out_avals), in_names=tuple(all_in),
            out_names=tuple(out_names), lowering_input_output_aliases=(),
            sim_require_finite=True, sim_require_nnan=True, nc=nc)
        return tuple(outs)

    devices = jax.devices()[:n_cores]
    mesh = Mesh(np.asarray(devices), ("core",))
    nin = n_params + len(out_names)
    sharded = jax.jit(
        shard_map(_body, mesh=mesh, in_specs=(PartitionSpec("core"),) * nin,
                  out_specs=(PartitionSpec("core"),) * len(out_names),
                  check_rep=False),
        keep_unused=True)
    concat_in = [np.concatenate([np.asarray(in_maps[c][nm]) for c in range(n_cores)], axis=0)
                 for nm in in_names]
    concat_zero = [np.zeros((n_cores * z.shape[0], *z.shape[1:]), z.dtype) for z in zero_outs]
    dev_in = [jax.device_put(a) for a in concat_in + concat_zero]
    return sharded, dev_in


def time_kernel(inputs, reps=16, program_reps=None):
    if program_reps is None:
        program_reps = int(os.environ.get("KERNEL_PROGRAM_REPS", "33"))
    """Estimate per-batch (64-image) exec ns via repeat-program differencing:
    exec = (T(program_reps) - T(1)) / (program_reps - 1); RPC overheads cancel.
    All reps write the same output region (out tensor does not scale with
    program_reps) so the differencing isolates device execution instead of
    host-side zero-buffer shipping."""
    global ALIAS_OUT
    import jax
    import time as _t
    from concourse import bass2jax

    bass2jax.install_neuronx_cc_hook()
    ALIAS_OUT = True
    for k in list(_NC_CACHE):
        if k.startswith("nc"):
            del _NC_CACHE[k]
    input_y = np.ascontiguousarray(np.asarray(inputs["input_y"]), dtype=np.float32)
    input_cb = np.ascontiguousarray(np.asarray(inputs["input_cb"]), dtype=np.float32)
    input_cr = np.ascontiguousarray(np.asarray(inputs["input_cr"]), dtype=np.float32)
    q = np.asarray(inputs["jpeg_quality"], dtype=np.float32)
    qt_y = np.asarray(inputs["quantization_table_y"], dtype=np.float32).reshape(8, 8)
    qt_c = np.asarray(inputs["quantization_table_c"], dtype=np.float32).reshape(8, 8)
    in_maps = _prep_in_maps(input_y, input_cb, input_cr, q, qt_y, qt_c)

    def bench(prog_reps):
        key = f"nc{prog_reps}"
        if key not in _NC_CACHE:
            _NC_CACHE[key] = build_nc(reps=prog_reps)
        sharded, dev_in = _make_sharded(_NC_CACHE[key], in_maps)
        jax.block_until_ready(sharded(*dev_in))  # warm
        times = []
        for _ in range(reps):
            t0 = _t.time()
            jax.block_until_ready(sharded(*dev_in))
            times.append(_t.time() - t0)
        return min(times), sorted(times)[len(times) // 2]

    t1_min, t1_med = bench(1)
    tR_min, tR_med = bench(program_reps)
    per_min = (tR_min - t1_min) / (program_reps - 1)
    per_med = (tR_med - t1_med) / (program_reps - 1)
    print(f"  T(1) min/med: {t1_min*1e3:.2f}/{t1_med*1e3:.2f} ms; "
          f"T({program_reps}) min/med: {tR_min*1e3:.2f}/{tR_med*1e3:.2f} ms")
    print(f"  per-batch exec: min-diff {per_min*1e6:.1f} us, med-diff {per_med*1e6:.1f} us")
    return per_med * 1e9


if __name__ == "__main__":
    rng = np.random.default_rng(0)
    B = 64
    inputs = dict(
        input_y=(rng.standard_normal((B, 4096, 8, 8)) * 10).astype(np.float32),
        input_cb=(rng.standard_normal((B, 1024, 8, 8)) * 10).astype(np.float32),
        input_cr=(rng.standard_normal((B, 1024, 8, 8)) * 10).astype(np.float32),
        jpeg_quality=rng.uniform(10, 95, size=B).astype(np.float32),
        quantization_table_y=QT_Y[None],
        quantization_table_c=QT_C[None],
        H=512, W=512,
    )
    out = kernel(**inputs)
    print("out", out.shape, out.dtype, float(np.abs(out).max()))

